# revision 16
# baseline (speedup 1.0000x reference)
"""Trainium2 Bass kernel for a relational GCN layer (message passing + LSTM + MLP).

Math (reference):
  S = feat[src]                               # [E, D] gather
  msgs[e] = edge_nn(S[e], W_rel[rel[e]])      # tied 2-layer relu MLP per relation
  agg = segment_sum(msgs, dst, N)             # [N, D]
  hn = LSTM-step(agg) from zero state         # [N, D]
  out = MLP(hn)                               # [N, D_OUT]

Key algebraic optimizations:
  1. msgs[e] depends only on (rel[e], src[e]): precompute the transformed node
     table H[r, s] = edge_nn(feat[s], W_rel[r]) once (dense GEMMs), then the
     edge phase is row-gather + weighted segment-sum.
  2. The SWDGE gather descriptor generation (~9 ns/row serialized on the
     GPSIMD engine) is the hard floor, so gathered rows are DEDUPLICATED per
     (dst-block of 625 nodes, src-bucket): each distinct (rel, src) is pulled
     once per block; edge multiplicity moves into a host-precomputed fp8
     count-selector matrix sel[slot, dst_local] streamed from HBM.
  3. The segment-sum matmul runs transposed: aggT[feat, dst] += msup^T @ sel
     (gathered tile stationary, 640-wide selector moving), so aggregates land
     feature-major and the LSTM/MLP phase needs no transposes and gets
     per-partition biases for free via the ACT unit.

Distribution: edges are sharded by DESTINATION node range across the 8 cores
(core c owns dst in [1250c, 1250(c+1))): every core computes complete
aggregates for its own 1250 nodes, no cross-core communication.

Pipeline: H is built in 5 src-buckets of growing size; the gather chain for
bucket q starts as soon as bucket q's H is in HBM.  Gathers run in sub-groups
of <=16 tiles; per-core valid counts are passed via a GPSIMD register
(num_idxs_reg) with trailing -1 index padding so cross-core padding costs no
descriptor-generation time.
"""

import math
import numpy as np
import ml_dtypes

import concourse.bacc as bacc
import concourse.bass as bass
import concourse.mybir as mybir
import concourse.tile as tile
from concourse import bass_utils
from concourse.tile import add_dep_helper

# ---- problem constants (hardcoded per spec) ----
N_NODES = 10000
N_EDGES = 320000
D = 256
D_OUT = 256
NUM_REL = 2
NCORES = 8
NPC = N_NODES // NCORES          # 1250 nodes per core
BLK = 625                        # dst-block size (2 blocks per core)
NBLK = 2
SELW = 640                       # sel matrix width (625 padded to 128-mult)
SUB = 16                         # max tiles per dma_gather sub-group
NPAD = 10240                     # node count padded to 20 tiles of 512
BUCKETS = [512, 1024, 2048, 3072, 3584]
BUCKET_BASE = [0, 512, 1536, 3584, 6656]
NQ = len(BUCKETS)

f32 = mybir.dt.float32
bf16 = mybir.dt.bfloat16
fp8 = mybir.dt.float8e4
u32 = mybir.dt.uint32
i16 = mybir.dt.int16

np_bf16 = ml_dtypes.bfloat16
np_fp8 = mybir.dt.np(fp8)


# ----------------------------------------------------------------------------
# host-side preprocessing
# ----------------------------------------------------------------------------

def _prep_edges(src, dst, rel):
    """Dedup edges per (core, dst-block, src-bucket); build gather index
    bands, fp8 count-selector matrices, and per-sub-group valid counts with a
    static tiling common to all cores.

    Slot p of a group lands at msup[p % 128, tile_ofs + p // 128, :]; its
    gather index goes to idxs16[p % 16, tile_ofs*8 + (p//2048)*128 + (p%2048)//16]
    (sub-groups of SUB=16 tiles are separate dma_gather calls over column
    slices of the band).
    """
    src = src.astype(np.int64)
    dst = dst.astype(np.int64)
    rel = rel.astype(np.int64)
    base = np.asarray(BUCKET_BASE, dtype=np.int64)
    sizes = np.asarray(BUCKETS, dtype=np.int64)
    q = np.searchsorted(base, src, side="right") - 1
    core = dst // NPC
    loc = dst % NPC
    blk = (loc >= BLK).astype(np.int64)
    dstloc = loc - blk * BLK
    rowid = rel * sizes[q] + (src - base[q])          # < 2*3584 = 7168
    gid = (core * NBLK + blk) * NQ + q                # 0..79
    key = gid * 8192 + rowid

    uk, inv = np.unique(key, return_inverse=True)
    gid_u = uk >> 13
    rowid_u = uk & 8191
    # position of each unique slot within its gid (uk is sorted by key)
    grp_first = np.searchsorted(uk >> 13, np.arange(NCORES * NBLK * NQ))
    pos_u = np.arange(len(uk)) - grp_first[gid_u]
    n_per_gid = np.bincount(gid_u, minlength=NCORES * NBLK * NQ)

    # static tiles per (blk, q) group: max over cores, in (q, blk) order
    n_cbq = n_per_gid.reshape(NCORES, NBLK, NQ)
    tiles_bq = [[int(math.ceil(n_cbq[:, b, qq].max() / 128))
                 for b in range(NBLK)] for qq in range(NQ)]
    grp_ofs = {}
    acc = 0
    subs = []            # static list of (q, b, tile_ofs, tb) sub-groups
    for qq in range(NQ):
        for b in range(NBLK):
            grp_ofs[qq, b] = acc
            t = tiles_bq[qq][b]
            s0 = 0
            while s0 < t:
                tb = min(SUB, t - s0)
                subs.append((qq, b, acc + s0, tb))
                s0 += tb
            acc += t
    nt = acc
    nsub = len(subs)

    idxs16_all = np.full((NCORES, 16, nt * 8), -1, dtype=np.int16)
    sel_i = np.zeros((NCORES, 128, nt, SELW), dtype=np.int16)
    cnts_all = np.zeros((NCORES, 1, nsub), dtype=np.uint32)

    # scatter unique-slot gather indices
    core_u = gid_u // (NBLK * NQ)
    b_u = (gid_u // NQ) % NBLK
    q_u = gid_u % NQ
    ofs_u = np.array([grp_ofs[qq, b] for qq in range(NQ) for b in range(NBLK)]
                     ).reshape(NQ, NBLK)[q_u, b_u]
    col = ofs_u * 8 + (pos_u // 2048) * 128 + (pos_u % 2048) // 16
    idxs16_all[core_u, pos_u % 16, col] = rowid_u.astype(np.int16)

    # per-edge multiplicity into sel (int accumulate, then fp8: exact <= 16)
    tile_e = ofs_u[inv] + pos_u[inv] // 128
    np.add.at(sel_i, (core_u[inv], pos_u[inv] % 128, tile_e, dstloc), 1)
    assert sel_i.max() <= 16, sel_i.max()
    sel_all = sel_i.astype(np_fp8)

    # per-sub-group valid counts; empty sub-groups get one sentinel row 0.
    # The first 4 sub-gathers (one per msup ring buffer) gather their full
    # padded count (pad indices 0) so every ring buffer starts fully written
    # with finite data; later sub-gathers may leave stale-but-finite tails.
    for c in range(NCORES):
        for i, (qq, b, t0, tb) in enumerate(subs):
            s0 = (t0 - grp_ofs[qq, b]) * 128
            n = int(n_cbq[c, b, qq])
            cnt = max(0, min(n - s0, tb * 128))
            if i < 4:
                band = idxs16_all[c, :, t0 * 8:(t0 + tb) * 8]
                band[band < 0] = 0
                cnt = tb * 128
            elif cnt == 0:
                idxs16_all[c, 0, t0 * 8] = 0    # sentinel: gather row 0
                cnt = 1
            cnts_all[c, 0, i] = cnt

    idxs16_all = np.tile(idxs16_all, (1, 8, 1))   # replicate to 128 partitions
    return tuple(tuple(r) for r in tiles_bq), idxs16_all, sel_all, cnts_all


def _prep_weights(inputs):
    feat = np.asarray(inputs["feat"], dtype=np.float32)
    W_rel = np.asarray(inputs["W_rel"], dtype=np.float32)
    b_rel = np.asarray(inputs["b_rel"], dtype=np.float32)
    W_ih = np.asarray(inputs["W_ih"], dtype=np.float32)
    b_ih = np.asarray(inputs["b_ih"], dtype=np.float32)
    b_hh = np.asarray(inputs["b_hh"], dtype=np.float32)
    W1 = np.asarray(inputs["W1"], dtype=np.float32)
    b1 = np.asarray(inputs["b1"], dtype=np.float32)
    W2 = np.asarray(inputs["W2"], dtype=np.float32)
    b2 = np.asarray(inputs["b2"], dtype=np.float32)
    W3 = np.asarray(inputs["W3"], dtype=np.float32)
    b3 = np.asarray(inputs["b3"], dtype=np.float32)

    featT = np.zeros((D, NPAD), dtype=np.float32)
    featT[:, :N_NODES] = feat.T
    keep = np.r_[0:256, 512:1024]  # i, g, o gate rows (f unused: c0 = 0)
    com = {
        "featT": featT.astype(np_bf16),
        "W_rT": np.ascontiguousarray(np.transpose(W_rel, (0, 2, 1))).astype(np_bf16),
        "b_r_col": np.ascontiguousarray(b_rel[:, :, None]),                  # f32
        "b_r_rep2": np.ascontiguousarray(np.broadcast_to(
            np.tile(b_rel, (1, 2))[:, None, :], (NUM_REL, 128, 2 * D))).copy(),
        "W_ihT": np.ascontiguousarray(W_ih[keep, :].T).astype(np_bf16),      # [256,768]
        "b_g_col": np.ascontiguousarray(
            (b_ih + b_hh)[keep].reshape(6, 128, 1)),                         # f32
        "W1T": np.ascontiguousarray(W1.T).astype(np_bf16),                   # [256,128]
        "b1_col": np.ascontiguousarray(b1[:, None]),
        "W2T": np.ascontiguousarray(W2.T).astype(np_bf16),                   # [128,128]
        "b2_col": np.ascontiguousarray(b2[:, None]),
        "W3T": np.ascontiguousarray(W3.T).astype(np_bf16),                   # [128,256]
        "b3_col": np.ascontiguousarray(b3.reshape(2, 128, 1)),
    }
    return com


# ----------------------------------------------------------------------------
# kernel builder
# ----------------------------------------------------------------------------

def _build(tiles_bq):
    Relu = mybir.ActivationFunctionType.Relu
    Sig = mybir.ActivationFunctionType.Sigmoid
    Tanh = mybir.ActivationFunctionType.Tanh
    Add = mybir.AluOpType.add
    Max = mybir.AluOpType.max

    grp_ofs = {}
    acc = 0
    subs = []
    for qq in range(NQ):
        for b in range(NBLK):
            grp_ofs[qq, b] = acc
            t = tiles_bq[qq][b]
            s0 = 0
            while s0 < t:
                tb = min(SUB, t - s0)
                subs.append((qq, b, acc + s0, tb))
                s0 += tb
            acc += t
    nt = acc
    nsub = len(subs)
    subs_of = {}
    for i, (qq, b, t0, tb) in enumerate(subs):
        subs_of.setdefault((qq, b), []).append((i, t0, tb))

    nc = bacc.Bacc("TRN2", target_bir_lowering=False, debug=False,
                   dynamic_dma_scratch_size=65536)

    featT_d = nc.dram_tensor("featT", [D, NPAD], bf16, kind="ExternalInput")
    W_rT_d = nc.dram_tensor("W_rT", [NUM_REL, D, D], bf16, kind="ExternalInput")
    b_r_col_d = nc.dram_tensor("b_r_col", [NUM_REL, D, 1], f32, kind="ExternalInput")
    b_r_rep2_d = nc.dram_tensor("b_r_rep2", [NUM_REL, 128, 2 * D], f32, kind="ExternalInput")
    W_ihT_d = nc.dram_tensor("W_ihT", [D, 768], bf16, kind="ExternalInput")
    b_g_col_d = nc.dram_tensor("b_g_col", [6, 128, 1], f32, kind="ExternalInput")
    W1T_d = nc.dram_tensor("W1T", [D, 128], bf16, kind="ExternalInput")
    b1_col_d = nc.dram_tensor("b1_col", [128, 1], f32, kind="ExternalInput")
    W2T_d = nc.dram_tensor("W2T", [128, 128], bf16, kind="ExternalInput")
    b2_col_d = nc.dram_tensor("b2_col", [128, 1], f32, kind="ExternalInput")
    W3T_d = nc.dram_tensor("W3T", [128, D_OUT], bf16, kind="ExternalInput")
    b3_col_d = nc.dram_tensor("b3_col", [2, 128, 1], f32, kind="ExternalInput")
    idxs_d = nc.dram_tensor("idxs16", [128, nt * 8], i16, kind="ExternalInput")
    sel_d = nc.dram_tensor("sel", [128, nt, SELW], fp8, kind="ExternalInput")
    cnts_d = nc.dram_tensor("cnts", [1, nsub], u32, kind="ExternalInput")

    outT_d = nc.dram_tensor("outT", [D_OUT, NPC], f32, kind="ExternalOutput")

    H_d = [nc.dram_tensor(f"Htab{g}", [NUM_REL * BUCKETS[g], D], bf16)
           for g in range(NQ)]

    with tile.TileContext(nc) as tc:
        with (
            tc.tile_pool(name="const", bufs=1) as cp,
            tc.tile_pool(name="work", bufs=3) as wp,
            tc.tile_pool(name="aggpool", bufs=1) as ap_pool,
            tc.tile_pool(name="psA", bufs=2, space="PSUM") as psA,
            tc.tile_pool(name="psB", bufs=2, space="PSUM") as psB,
        ):
            # ---- gather tables on the gpsimd queue (first: gathers need them) ----
            idxs_sb = cp.tile([128, nt * 8], i16, tag="idxs")
            nc.gpsimd.dma_start(idxs_sb[:], idxs_d[:, :])
            cnts_sb = cp.tile([1, nsub], u32, tag="cnts")
            nc.gpsimd.dma_start(cnts_sb[:], cnts_d[:, :])

            # phase-A weights on the scalar queue (short; sync stays free for
            # featT streaming)
            W_rT_sb = {}
            for r in range(NUM_REL):
                for h in range(2):
                    t = cp.tile([128, D], bf16, tag=f"wrt{r}{h}")
                    nc.gpsimd.dma_start(t[:], W_rT_d[r, h * 128:(h + 1) * 128, :])
                    W_rT_sb[r, h] = t
            b_r_col_sb = {}
            for r in range(NUM_REL):
                for h in range(2):
                    t = cp.tile([128, 1], f32, tag=f"brc{r}{h}")
                    nc.gpsimd.dma_start(t[:], b_r_col_d[r, h * 128:(h + 1) * 128, :])
                    b_r_col_sb[r, h] = t
            b_r_rep2_sb = {}
            for r in range(NUM_REL):
                t = cp.tile([128, 2 * D], f32, tag=f"brr{r}")
                nc.gpsimd.dma_start(t[:], b_r_rep2_d[r, :, :])
                b_r_rep2_sb[r] = t

            # SBUF aggregate accumulators, transposed: [feat_half, 640 dst]
            aggT_sb = {}
            for b in range(NBLK):
                for fh in range(2):
                    aggT_sb[b, fh] = ap_pool.tile(
                        [128, SELW], f32, tag=f"agg{b}{fh}", name=f"agg{b}{fh}")

            # phase-C weights: emitted onto the sync queue after all phase-A
            # featT traffic (see schedule below); dict filled lazily
            CW = {}

            def load_phase_c_weights():
                for h in range(2):
                    t = cp.tile([128, 768], bf16, tag=f"wih{h}", name=f"wih{h}")
                    nc.sync.dma_start(t[:], W_ihT_d[h * 128:(h + 1) * 128, :])
                    CW["wih", h] = t
                for g in range(6):
                    t = cp.tile([128, 1], f32, tag=f"bg{g}", name=f"bg{g}")
                    nc.sync.dma_start(t[:], b_g_col_d[g, :, :])
                    CW["bg", g] = t
                for h in range(2):
                    t = cp.tile([128, 128], bf16, tag=f"w1t{h}", name=f"w1t{h}")
                    nc.sync.dma_start(t[:], W1T_d[h * 128:(h + 1) * 128, :])
                    CW["w1t", h] = t
                t = cp.tile([128, 1], f32, tag="b1", name="b1c")
                nc.sync.dma_start(t[:], b1_col_d[:, :])
                CW["b1"] = t
                t = cp.tile([128, 128], bf16, tag="w2t", name="w2t")
                nc.sync.dma_start(t[:], W2T_d[:, :])
                CW["w2t"] = t
                t = cp.tile([128, 1], f32, tag="b2", name="b2c")
                nc.sync.dma_start(t[:], b2_col_d[:, :])
                CW["b2"] = t
                t = cp.tile([128, D_OUT], bf16, tag="w3t", name="w3t")
                nc.sync.dma_start(t[:], W3T_d[:, :])
                CW["w3t"] = t
                for h in range(2):
                    t = cp.tile([128, 1], f32, tag=f"b3{h}", name=f"b3c{h}")
                    nc.sync.dma_start(t[:], b3_col_d[h, :, :])
                    CW["b3", h] = t

            # ---- phase A with featT prefetch 2 loops ahead (cross-quarter) ----
            all_loops = [(q, r, ntl) for q in range(NQ) for r in range(NUM_REL)
                         for ntl in range(BUCKETS[q] // 512)]
            loop_of = {}
            for j, (q, r, ntl) in enumerate(all_loops):
                loop_of[q, r, ntl] = j
            ft_tiles = {}
            ft_next = [0]

            def ensure_ft(upto):
                while ft_next[0] <= min(upto, len(all_loops) - 1):
                    j = ft_next[0]
                    q, r, ntl = all_loops[j]
                    c0 = BUCKET_BASE[q] + ntl * 512
                    pair = []
                    for h in range(2):
                        t = wp.tile([128, 512], bf16, tag=f"ft{h}", bufs=3,
                                    name=f"ft{h}_{j}")
                        nc.sync.dma_start(
                            t[:], featT_d[h * 128:(h + 1) * 128, c0:c0 + 512])
                        pair.append(t)
                    ft_tiles[j] = pair
                    ft_next[0] += 1

            h_writes = [[] for _ in range(NQ)]

            def phase_a(q, rels=(0, 1)):
                """Build H table for bucket q, given relations, write to HBM."""
                for r in rels:
                    for ntl in range(BUCKETS[q] // 512):
                        j = loop_of[q, r, ntl]
                        ensure_ft(j + 2)
                        ft = ft_tiles.pop(j)
                        z1s = {}
                        for do_h in range(2):
                            z1p = psA.tile([128, 512], f32, tag="z1",
                                           space="PSUM", bufs=2)
                            for di_h in range(2):
                                nc.tensor.matmul(
                                    z1p[:],
                                    lhsT=W_rT_sb[r, di_h][:, do_h * 128:(do_h + 1) * 128],
                                    rhs=ft[di_h][:],
                                    start=(di_h == 0), stop=(di_h == 1))
                            # per-partition bias + relu + cast on the ACT unit
                            z = wp.tile([128, 512], bf16, tag=f"z1s{do_h}")
                            nc.scalar.activation(z[:], z1p[:], Relu,
                                                 bias=b_r_col_sb[r, do_h][:],
                                                 scale=1.0)
                            z1s[do_h] = z
                        hs = wp.tile([128, 4, D], bf16, tag="hs", bufs=2)
                        for cp2 in range(2):     # pairs of 128-node chunks
                            hp = psA.tile([128, 512], f32, tag="hp",
                                          space="PSUM", bufs=2)
                            for c4 in range(2):
                                sl = slice((cp2 * 2 + c4) * 128,
                                           (cp2 * 2 + c4 + 1) * 128)
                                dst_sl = slice(c4 * 256, (c4 + 1) * 256)
                                nc.tensor.matmul(hp[:, dst_sl],
                                                 lhsT=z1s[0][:, sl],
                                                 rhs=W_rT_sb[r, 0][:],
                                                 start=True, stop=False)
                                nc.tensor.matmul(hp[:, dst_sl],
                                                 lhsT=z1s[1][:, sl],
                                                 rhs=W_rT_sb[r, 1][:],
                                                 start=False, stop=True)
                            # bias lives on the free dim: DVE add (bf16 out),
                            # then ACT relu straight into the H staging tile
                            hpt = wp.tile([128, 512], bf16, tag="hpt", bufs=2)
                            nc.vector.tensor_tensor(out=hpt[:], in0=hp[:],
                                                    in1=b_r_rep2_sb[r][:],
                                                    op=Add)
                            nc.scalar.activation(hs[:, cp2 * 2:cp2 * 2 + 2, :],
                                                 hpt[:], Relu, bias=0.0,
                                                 scale=1.0)
                        row0 = r * BUCKETS[q] + ntl * 512
                        w = nc.sync.dma_start(
                            H_d[q][row0:row0 + 512, :].rearrange(
                                "(c p) d -> p c d", p=128),
                            hs[:])
                        h_writes[q].append(w.ins)

            cnt_reg = nc.gpsimd.alloc_register("cnt_reg")

            def gather_sub(i, q, t0, tb):
                """One dma_gather sub-group; returns (msup, sel)."""
                sel_t = wp.tile([128, SUB, SELW], fp8, tag="sel", bufs=3,
                                name=f"sel{i}")
                nc.sync.dma_start(sel_t[:, 0:tb, :], sel_d[:, t0:t0 + tb, :])
                msup = wp.tile([128, SUB, D], bf16, tag="msup", bufs=4)
                nc.gpsimd.reg_load(cnt_reg, cnts_sb[:, i:i + 1])
                g_inst = nc.gpsimd.dma_gather(
                    out_ap=msup[:, 0:tb, :], in_ap=H_d[q][:],
                    idxs_ap=idxs_sb[:, t0 * 8:(t0 + tb) * 8],
                    num_idxs=tb * 128, num_idxs_reg=cnt_reg,
                    elem_size=D, single_packet=False)
                for w in h_writes[q]:
                    add_dep_helper(g_inst.ins, w,
                                   reason="gather waits on Htab writes")
                return msup, sel_t

            def seg_sum(q, b):
                """Weighted segment-sum for group (q, b): aggT += msup^T @ sel."""
                first = (q == 0)
                pb = {}
                for fh in range(2):
                    pb[fh, 0] = psB.tile([128, 512], f32, tag="pb512",
                                         space="PSUM", bufs=2, name=f"pb512_{fh}")
                    pb[fh, 1] = psB.tile([128, 128], f32, tag="pb128",
                                         space="PSUM", bufs=2, name=f"pb128_{fh}")
                sub_list = subs_of[q, b]
                n_tiles = sum(tb for _, _, tb in sub_list)
                done = 0
                for i, t0, tb in sub_list:
                    msup, sel_t = gather_sub(i, q, t0, tb)
                    for t in range(tb):
                        st = (done + t == 0)
                        sp = (done + t == n_tiles - 1)
                        for fh in range(2):
                            lhsT = msup[:, t, fh * 128:(fh + 1) * 128]
                            nc.tensor.matmul(pb[fh, 0][:], lhsT=lhsT,
                                             rhs=sel_t[:, t, 0:512],
                                             start=st, stop=sp)
                            nc.tensor.matmul(pb[fh, 1][:], lhsT=lhsT,
                                             rhs=sel_t[:, t, 512:SELW],
                                             start=st, stop=sp)
                    done += tb
                for fh in range(2):
                    if first:
                        nc.vector.tensor_copy(aggT_sb[b, fh][:, 0:512],
                                              pb[fh, 0][:])
                        nc.vector.tensor_copy(aggT_sb[b, fh][:, 512:SELW],
                                              pb[fh, 1][:])
                    else:
                        nc.vector.tensor_add(aggT_sb[b, fh][:, 0:512],
                                             aggT_sb[b, fh][:, 0:512], pb[fh, 0][:])
                        nc.vector.tensor_add(aggT_sb[b, fh][:, 512:SELW],
                                             aggT_sb[b, fh][:, 512:SELW], pb[fh, 1][:])

            def phase_c(b):
                """LSTM step + MLP for block b, transposed layout, batched."""
                n0 = b * BLK
                nn = min(BLK, NPC - n0)
                aggT = {}
                for fh in range(2):
                    t = wp.tile([128, SELW], bf16, tag=f"aggb{fh}", bufs=2)
                    nc.vector.tensor_copy(t[:], aggT_sb[b, fh][:])
                    aggT[fh] = t
                for ck, (cs, cw) in enumerate(((0, 512), (512, 128))):
                    if cs >= nn:
                        continue
                    cw_v = min(cw, nn - cs)    # valid columns this chunk
                    gate_sb = {}
                    for gi, gname, fn in ((0, "i", Sig), (1, "g", Tanh),
                                          (2, "o", Sig)):
                        for gh in range(2):
                            gp = psA.tile([128, 512], f32, tag="z1",
                                          space="PSUM", bufs=2)
                            gc = gi * 2 + gh
                            for h in range(2):
                                nc.tensor.matmul(
                                    gp[:, 0:cw],
                                    lhsT=CW["wih", h][:, gc * 128:(gc + 1) * 128],
                                    rhs=aggT[h][:, cs:cs + cw],
                                    start=(h == 0), stop=(h == 1))
                            a = wp.tile([128, 512], f32, tag=f"gact{gh}", bufs=2)
                            nc.scalar.activation(a[:, 0:cw], gp[:, 0:cw], fn,
                                                 bias=CW["bg", gc][:], scale=1.0)
                            gate_sb[gname, gh] = a
                    hnT = {}
                    for gh in range(2):
                        cc = wp.tile([128, 512], f32, tag=f"cc{gh}", bufs=2)
                        nc.vector.tensor_mul(cc[:, 0:cw], gate_sb["i", gh][:, 0:cw],
                                             gate_sb["g", gh][:, 0:cw])
                        nc.scalar.activation(cc[:, 0:cw], cc[:, 0:cw], Tanh,
                                             bias=0.0, scale=1.0)
                        hn = wp.tile([128, 512], bf16, tag=f"hn{gh}", bufs=2)
                        nc.vector.tensor_mul(hn[:, 0:cw], gate_sb["o", gh][:, 0:cw],
                                             cc[:, 0:cw])
                        hnT[gh] = hn
                    # MLP: x1 = relu(W1 hn + b1), x2 = relu(W2 x1 + b2)
                    x1p = psA.tile([128, 512], f32, tag="z1",
                                   space="PSUM", bufs=2)
                    for h in range(2):
                        nc.tensor.matmul(x1p[:, 0:cw], lhsT=CW["w1t", h][:],
                                         rhs=hnT[h][:, 0:cw],
                                         start=(h == 0), stop=(h == 1))
                    x1s = wp.tile([128, 512], bf16, tag="x1s", bufs=2)
                    nc.scalar.activation(x1s[:, 0:cw], x1p[:, 0:cw], Relu,
                                         bias=CW["b1"][:], scale=1.0)
                    x2p = psA.tile([128, 512], f32, tag="z1",
                                   space="PSUM", bufs=2)
                    nc.tensor.matmul(x2p[:, 0:cw], lhsT=CW["w2t"][:],
                                     rhs=x1s[:, 0:cw], start=True, stop=True)
                    x2s = wp.tile([128, 512], bf16, tag="x2s", bufs=2)
                    nc.scalar.activation(x2s[:, 0:cw], x2p[:, 0:cw], Relu,
                                         bias=CW["b2"][:], scale=1.0)
                    for oh in range(2):
                        op = psA.tile([128, 512], f32, tag="z1",
                                      space="PSUM", bufs=2)
                        nc.tensor.matmul(op[:, 0:cw],
                                         lhsT=CW["w3t"][:, oh * 128:(oh + 1) * 128],
                                         rhs=x2s[:, 0:cw], start=True, stop=True)
                        osb = wp.tile([128, 512], f32, tag=f"osb{oh}", bufs=1)
                        nc.vector.tensor_scalar_add(
                            osb[:, 0:cw], op[:, 0:cw], CW["b3", oh][:])
                        nc.sync.dma_start(
                            outT_d[oh * 128:(oh + 1) * 128,
                                   n0 + cs:n0 + cs + cw_v],
                            osb[:, 0:cw_v])

            # ---- schedule: interleave phase A (producer) with phase B at
            # per-relation granularity so the PE never traps the H producer
            # behind long consumer batches ----
            phase_a(0)
            phase_a(1)
            seg_sum(0, 0)
            phase_a(2, rels=(0,))
            seg_sum(0, 1)
            phase_a(2, rels=(1,))
            seg_sum(1, 0)
            phase_a(3, rels=(0,))
            seg_sum(1, 1)
            phase_a(3, rels=(1,))
            seg_sum(2, 0)
            phase_a(4, rels=(0,))
            seg_sum(2, 1)
            phase_a(4, rels=(1,))
            load_phase_c_weights()
            seg_sum(3, 0)
            seg_sum(3, 1)
            seg_sum(4, 0)
            phase_c(0)
            seg_sum(4, 1)
            phase_c(1)

    nc.compile()
    return nc


_CACHE = {}


def _get_nc(tiles_key):
    if tiles_key not in _CACHE:
        _CACHE[tiles_key] = _build([list(r) for r in tiles_key])
    return _CACHE[tiles_key]


# ----------------------------------------------------------------------------
# public entry
# ----------------------------------------------------------------------------

def kernel(**inputs) -> np.ndarray:
    src = np.asarray(inputs["src"], dtype=np.int32)
    dst = np.asarray(inputs["dst"], dtype=np.int32)
    rel = np.asarray(inputs["rel"], dtype=np.int32)

    com = _prep_weights(inputs)
    tiles_bq, idxs16_all, sel_all, cnts_all = _prep_edges(src, dst, rel)

    nc = _get_nc(tiles_bq)

    in_maps = []
    for c in range(NCORES):
        m = dict(com)
        m["idxs16"] = np.ascontiguousarray(idxs16_all[c])
        m["sel"] = np.ascontiguousarray(sel_all[c])
        m["cnts"] = np.ascontiguousarray(cnts_all[c])
        in_maps.append(m)

    res = bass_utils.run_bass_kernel_spmd(nc, in_maps, core_ids=list(range(NCORES)))

    out = np.empty((N_NODES, D_OUT), dtype=np.float32)
    for c in range(NCORES):
        out[c * NPC:(c + 1) * NPC, :] = res.results[c]["outT"].T
    return out


# revision 19
# speedup vs baseline: 1.0276x; 1.0276x over previous
"""Trainium2 Bass kernel for a relational GCN layer (message passing + LSTM + MLP).

Math (reference):
  S = feat[src]                               # [E, D] gather
  msgs[e] = edge_nn(S[e], W_rel[rel[e]])      # tied 2-layer relu MLP per relation
  agg = segment_sum(msgs, dst, N)             # [N, D]
  hn = LSTM-step(agg) from zero state         # [N, D]
  out = MLP(hn)                               # [N, D_OUT]

Key algebraic optimizations:
  1. msgs[e] depends only on (rel[e], src[e]): precompute the transformed node
     table H[r, s] = edge_nn(feat[s], W_rel[r]) once (dense GEMMs), then the
     edge phase is row-gather + weighted segment-sum.
  2. The SWDGE gather descriptor generation (~9 ns/row serialized on the
     GPSIMD engine) is the hard floor, so gathered rows are DEDUPLICATED per
     (dst-block of 625 nodes, src-bucket): each distinct (rel, src) is pulled
     once per block; edge multiplicity moves into a host-precomputed fp8
     count-selector matrix sel[slot, dst_local] streamed from HBM.
  3. The segment-sum matmul runs transposed: aggT[feat, dst] += msup^T @ sel
     (gathered tile stationary, 640-wide selector moving), so aggregates land
     feature-major and the LSTM/MLP phase needs no transposes and gets
     per-partition biases for free via the ACT unit.

Distribution: edges are sharded by DESTINATION node range across the 8 cores
(core c owns dst in [1250c, 1250(c+1))): every core computes complete
aggregates for its own 1250 nodes, no cross-core communication.

Pipeline: H is built in 5 src-buckets of growing size; the gather chain for
bucket q starts as soon as bucket q's H is in HBM.  Gathers run in sub-groups
of <=16 tiles; per-core valid counts are passed via a GPSIMD register
(num_idxs_reg) with trailing -1 index padding so cross-core padding costs no
descriptor-generation time.
"""

import math
import numpy as np
import ml_dtypes

import concourse.bacc as bacc
import concourse.bass as bass
import concourse.mybir as mybir
import concourse.tile as tile
from concourse import bass_utils
from concourse.tile import add_dep_helper

# ---- problem constants (hardcoded per spec) ----
N_NODES = 10000
N_EDGES = 320000
D = 256
D_OUT = 256
NUM_REL = 2
NCORES = 8
NPC = N_NODES // NCORES          # 1250 nodes per core
BLK = 625                        # dst-block size (2 blocks per core)
NBLK = 2
SELW = 640                       # sel matrix width (625 padded to 128-mult)
SUB = 16                         # max tiles per dma_gather sub-group
NPAD = 10240                     # node count padded to 20 tiles of 512
BUCKETS = [512, 1024, 2048, 3072, 3584]
BUCKET_BASE = [0, 512, 1536, 3584, 6656]
NQ = len(BUCKETS)

f32 = mybir.dt.float32
bf16 = mybir.dt.bfloat16
fp8 = mybir.dt.float8e4
u32 = mybir.dt.uint32
i16 = mybir.dt.int16

np_bf16 = ml_dtypes.bfloat16
np_fp8 = mybir.dt.np(fp8)


# ----------------------------------------------------------------------------
# host-side preprocessing
# ----------------------------------------------------------------------------

def _prep_edges(src, dst, rel):
    """Dedup edges per (core, dst-block, src-bucket); build gather index
    bands, fp8 count-selector matrices, and per-sub-group valid counts with a
    static tiling common to all cores.

    Slot p of a group lands at msup[p % 128, tile_ofs + p // 128, :]; its
    gather index goes to idxs16[p % 16, tile_ofs*8 + (p//2048)*128 + (p%2048)//16]
    (sub-groups of SUB=16 tiles are separate dma_gather calls over column
    slices of the band).
    """
    src = src.astype(np.int64)
    dst = dst.astype(np.int64)
    rel = rel.astype(np.int64)
    base = np.asarray(BUCKET_BASE, dtype=np.int64)
    sizes = np.asarray(BUCKETS, dtype=np.int64)
    q = np.searchsorted(base, src, side="right") - 1
    core = dst // NPC
    loc = dst % NPC
    blk = (loc >= BLK).astype(np.int64)
    dstloc = loc - blk * BLK
    rowid = rel * sizes[q] + (src - base[q])          # < 2*3584 = 7168
    gid = (core * NBLK + blk) * NQ + q                # 0..79
    key = gid * 8192 + rowid

    uk, inv = np.unique(key, return_inverse=True)
    gid_u = uk >> 13
    rowid_u = uk & 8191
    # position of each unique slot within its gid (uk is sorted by key)
    grp_first = np.searchsorted(uk >> 13, np.arange(NCORES * NBLK * NQ))
    pos_u = np.arange(len(uk)) - grp_first[gid_u]
    n_per_gid = np.bincount(gid_u, minlength=NCORES * NBLK * NQ)

    # static tiles per (blk, q) group: max over cores, in (q, blk) order
    n_cbq = n_per_gid.reshape(NCORES, NBLK, NQ)
    tiles_bq = [[int(math.ceil(n_cbq[:, b, qq].max() / 128))
                 for b in range(NBLK)] for qq in range(NQ)]
    grp_ofs = {}
    acc = 0
    subs = []            # static list of (q, b, tile_ofs, tb) sub-groups
    for qq in range(NQ):
        for b in range(NBLK):
            grp_ofs[qq, b] = acc
            t = tiles_bq[qq][b]
            s0 = 0
            while s0 < t:
                tb = min(SUB, t - s0)
                subs.append((qq, b, acc + s0, tb))
                s0 += tb
            acc += t
    nt = acc
    nsub = len(subs)

    idxs16_all = np.full((NCORES, 16, nt * 8), -1, dtype=np.int16)
    sel_i = np.zeros((NCORES, 128, nt, SELW), dtype=np.int16)
    cnts_all = np.zeros((NCORES, 1, nsub), dtype=np.uint32)

    # scatter unique-slot gather indices
    core_u = gid_u // (NBLK * NQ)
    b_u = (gid_u // NQ) % NBLK
    q_u = gid_u % NQ
    ofs_u = np.array([grp_ofs[qq, b] for qq in range(NQ) for b in range(NBLK)]
                     ).reshape(NQ, NBLK)[q_u, b_u]
    col = ofs_u * 8 + (pos_u // 2048) * 128 + (pos_u % 2048) // 16
    idxs16_all[core_u, pos_u % 16, col] = rowid_u.astype(np.int16)

    # per-edge multiplicity into sel (int accumulate, then fp8: exact <= 16)
    tile_e = ofs_u[inv] + pos_u[inv] // 128
    np.add.at(sel_i, (core_u[inv], pos_u[inv] % 128, tile_e, dstloc), 1)
    assert sel_i.max() <= 16, sel_i.max()
    sel_all = sel_i.astype(np_fp8)

    # per-sub-group valid counts; empty sub-groups get one sentinel row 0.
    # The first 4 sub-gathers (one per msup ring buffer) gather their full
    # padded count (pad indices 0) so every ring buffer starts fully written
    # with finite data; later sub-gathers may leave stale-but-finite tails.
    for c in range(NCORES):
        for i, (qq, b, t0, tb) in enumerate(subs):
            s0 = (t0 - grp_ofs[qq, b]) * 128
            n = int(n_cbq[c, b, qq])
            cnt = max(0, min(n - s0, tb * 128))
            if i < 4:
                band = idxs16_all[c, :, t0 * 8:(t0 + tb) * 8]
                band[band < 0] = 0
                cnt = tb * 128
            elif cnt == 0:
                idxs16_all[c, 0, t0 * 8] = 0    # sentinel: gather row 0
                cnt = 1
            cnts_all[c, 0, i] = cnt

    idxs16_all = np.tile(idxs16_all, (1, 8, 1))   # replicate to 128 partitions
    return tuple(tuple(r) for r in tiles_bq), idxs16_all, sel_all, cnts_all


def _prep_weights(inputs):
    feat = np.asarray(inputs["feat"], dtype=np.float32)
    W_rel = np.asarray(inputs["W_rel"], dtype=np.float32)
    b_rel = np.asarray(inputs["b_rel"], dtype=np.float32)
    W_ih = np.asarray(inputs["W_ih"], dtype=np.float32)
    b_ih = np.asarray(inputs["b_ih"], dtype=np.float32)
    b_hh = np.asarray(inputs["b_hh"], dtype=np.float32)
    W1 = np.asarray(inputs["W1"], dtype=np.float32)
    b1 = np.asarray(inputs["b1"], dtype=np.float32)
    W2 = np.asarray(inputs["W2"], dtype=np.float32)
    b2 = np.asarray(inputs["b2"], dtype=np.float32)
    W3 = np.asarray(inputs["W3"], dtype=np.float32)
    b3 = np.asarray(inputs["b3"], dtype=np.float32)

    featT = np.zeros((D, NPAD), dtype=np.float32)
    featT[:, :N_NODES] = feat.T
    keep = np.r_[0:256, 512:1024]  # i, g, o gate rows (f unused: c0 = 0)
    com = {
        "featT": featT.astype(np_bf16),
        "W_rT": np.ascontiguousarray(np.transpose(W_rel, (0, 2, 1))).astype(np_bf16),
        "b_r_col": np.ascontiguousarray(b_rel[:, :, None]),                  # f32
        "b_r_rep2": np.ascontiguousarray(np.broadcast_to(
            np.tile(b_rel, (1, 2))[:, None, :], (NUM_REL, 128, 2 * D))).copy(),
        "W_ihT": np.ascontiguousarray(W_ih[keep, :].T).astype(np_bf16),      # [256,768]
        "b_g_col": np.ascontiguousarray(
            (b_ih + b_hh)[keep].reshape(6, 128, 1)),                         # f32
        "W1T": np.ascontiguousarray(W1.T).astype(np_bf16),                   # [256,128]
        "b1_col": np.ascontiguousarray(b1[:, None]),
        "W2T": np.ascontiguousarray(W2.T).astype(np_bf16),                   # [128,128]
        "b2_col": np.ascontiguousarray(b2[:, None]),
        "W3T": np.ascontiguousarray(W3.T).astype(np_bf16),                   # [128,256]
        "b3_col": np.ascontiguousarray(b3.reshape(2, 128, 1)),
    }
    return com


# ----------------------------------------------------------------------------
# kernel builder
# ----------------------------------------------------------------------------

def _build(tiles_bq):
    Relu = mybir.ActivationFunctionType.Relu
    Sig = mybir.ActivationFunctionType.Sigmoid
    Tanh = mybir.ActivationFunctionType.Tanh
    Add = mybir.AluOpType.add
    Max = mybir.AluOpType.max

    grp_ofs = {}
    acc = 0
    subs = []
    for qq in range(NQ):
        for b in range(NBLK):
            grp_ofs[qq, b] = acc
            t = tiles_bq[qq][b]
            s0 = 0
            while s0 < t:
                tb = min(SUB, t - s0)
                subs.append((qq, b, acc + s0, tb))
                s0 += tb
            acc += t
    nt = acc
    nsub = len(subs)
    subs_of = {}
    for i, (qq, b, t0, tb) in enumerate(subs):
        subs_of.setdefault((qq, b), []).append((i, t0, tb))

    nc = bacc.Bacc("TRN2", target_bir_lowering=False, debug=False,
                   dynamic_dma_scratch_size=65536)

    featT_d = nc.dram_tensor("featT", [D, NPAD], bf16, kind="ExternalInput")
    W_rT_d = nc.dram_tensor("W_rT", [NUM_REL, D, D], bf16, kind="ExternalInput")
    b_r_col_d = nc.dram_tensor("b_r_col", [NUM_REL, D, 1], f32, kind="ExternalInput")
    b_r_rep2_d = nc.dram_tensor("b_r_rep2", [NUM_REL, 128, 2 * D], f32, kind="ExternalInput")
    W_ihT_d = nc.dram_tensor("W_ihT", [D, 768], bf16, kind="ExternalInput")
    b_g_col_d = nc.dram_tensor("b_g_col", [6, 128, 1], f32, kind="ExternalInput")
    W1T_d = nc.dram_tensor("W1T", [D, 128], bf16, kind="ExternalInput")
    b1_col_d = nc.dram_tensor("b1_col", [128, 1], f32, kind="ExternalInput")
    W2T_d = nc.dram_tensor("W2T", [128, 128], bf16, kind="ExternalInput")
    b2_col_d = nc.dram_tensor("b2_col", [128, 1], f32, kind="ExternalInput")
    W3T_d = nc.dram_tensor("W3T", [128, D_OUT], bf16, kind="ExternalInput")
    b3_col_d = nc.dram_tensor("b3_col", [2, 128, 1], f32, kind="ExternalInput")
    idxs_d = nc.dram_tensor("idxs16", [128, nt * 8], i16, kind="ExternalInput")
    sel_d = nc.dram_tensor("sel", [128, nt, SELW], fp8, kind="ExternalInput")
    cnts_d = nc.dram_tensor("cnts", [1, nsub], u32, kind="ExternalInput")

    outT_d = nc.dram_tensor("outT", [D_OUT, NPC], f32, kind="ExternalOutput")

    H_d = [nc.dram_tensor(f"Htab{g}", [NUM_REL * BUCKETS[g], D], bf16)
           for g in range(NQ)]

    with tile.TileContext(nc) as tc:
        with (
            tc.tile_pool(name="const", bufs=1) as cp,
            tc.tile_pool(name="work", bufs=3) as wp,
            tc.tile_pool(name="aggpool", bufs=1) as ap_pool,
            tc.tile_pool(name="psA", bufs=2, space="PSUM") as psA,
            tc.tile_pool(name="psB", bufs=2, space="PSUM") as psB,
        ):
            # ---- gather tables on the gpsimd queue (first: gathers need them) ----
            idxs_sb = cp.tile([128, nt * 8], i16, tag="idxs")
            nc.gpsimd.dma_start(idxs_sb[:], idxs_d[:, :])
            cnts_sb = cp.tile([1, nsub], u32, tag="cnts")
            nc.gpsimd.dma_start(cnts_sb[:], cnts_d[:, :])

            # phase-A weights on the scalar queue (short; sync stays free for
            # featT streaming)
            W_rT_sb = {}
            for r in range(NUM_REL):
                for h in range(2):
                    t = cp.tile([128, D], bf16, tag=f"wrt{r}{h}")
                    nc.gpsimd.dma_start(t[:], W_rT_d[r, h * 128:(h + 1) * 128, :])
                    W_rT_sb[r, h] = t
            b_r_col_sb = {}
            for r in range(NUM_REL):
                for h in range(2):
                    t = cp.tile([128, 1], f32, tag=f"brc{r}{h}")
                    nc.gpsimd.dma_start(t[:], b_r_col_d[r, h * 128:(h + 1) * 128, :])
                    b_r_col_sb[r, h] = t
            b_r_rep2_sb = {}
            for r in range(NUM_REL):
                t = cp.tile([128, 2 * D], f32, tag=f"brr{r}")
                nc.gpsimd.dma_start(t[:], b_r_rep2_d[r, :, :])
                b_r_rep2_sb[r] = t

            # zero the msup gather ring once: the trimmed gathers leave tail
            # slots (and whole tail tiles of short sub-groups) unwritten, and
            # 0 * garbage is only 0 when the stale bits are finite.  After
            # this, stale data is always old H rows.
            for k in range(4):
                t = wp.tile([128, SUB, D], bf16, tag="msup", bufs=4,
                            name=f"msup_init{k}")
                nc.vector.memset(t[:], 0.0)

            # SBUF aggregate accumulators, transposed: [feat_half, 640 dst]
            aggT_sb = {}
            for b in range(NBLK):
                for fh in range(2):
                    aggT_sb[b, fh] = ap_pool.tile(
                        [128, SELW], f32, tag=f"agg{b}{fh}", name=f"agg{b}{fh}")

            # phase-C weights: emitted onto the sync queue after all phase-A
            # featT traffic (see schedule below); dict filled lazily
            CW = {}

            def load_phase_c_weights():
                for h in range(2):
                    t = cp.tile([128, 768], bf16, tag=f"wih{h}", name=f"wih{h}")
                    nc.sync.dma_start(t[:], W_ihT_d[h * 128:(h + 1) * 128, :])
                    CW["wih", h] = t
                for g in range(6):
                    t = cp.tile([128, 1], f32, tag=f"bg{g}", name=f"bg{g}")
                    nc.sync.dma_start(t[:], b_g_col_d[g, :, :])
                    CW["bg", g] = t
                for h in range(2):
                    t = cp.tile([128, 128], bf16, tag=f"w1t{h}", name=f"w1t{h}")
                    nc.sync.dma_start(t[:], W1T_d[h * 128:(h + 1) * 128, :])
                    CW["w1t", h] = t
                t = cp.tile([128, 1], f32, tag="b1", name="b1c")
                nc.sync.dma_start(t[:], b1_col_d[:, :])
                CW["b1"] = t
                t = cp.tile([128, 128], bf16, tag="w2t", name="w2t")
                nc.sync.dma_start(t[:], W2T_d[:, :])
                CW["w2t"] = t
                t = cp.tile([128, 1], f32, tag="b2", name="b2c")
                nc.sync.dma_start(t[:], b2_col_d[:, :])
                CW["b2"] = t
                t = cp.tile([128, D_OUT], bf16, tag="w3t", name="w3t")
                nc.sync.dma_start(t[:], W3T_d[:, :])
                CW["w3t"] = t
                for h in range(2):
                    t = cp.tile([128, 1], f32, tag=f"b3{h}", name=f"b3c{h}")
                    nc.sync.dma_start(t[:], b3_col_d[h, :, :])
                    CW["b3", h] = t

            # ---- phase A with featT prefetch 2 loops ahead (cross-quarter) ----
            all_loops = [(q, r, ntl) for q in range(NQ) for r in range(NUM_REL)
                         for ntl in range(BUCKETS[q] // 512)]
            loop_of = {}
            for j, (q, r, ntl) in enumerate(all_loops):
                loop_of[q, r, ntl] = j
            ft_tiles = {}
            ft_next = [0]

            def ensure_ft(upto):
                while ft_next[0] <= min(upto, len(all_loops) - 1):
                    j = ft_next[0]
                    q, r, ntl = all_loops[j]
                    c0 = BUCKET_BASE[q] + ntl * 512
                    pair = []
                    for h in range(2):
                        t = wp.tile([128, 512], bf16, tag=f"ft{h}", bufs=3,
                                    name=f"ft{h}_{j}")
                        nc.sync.dma_start(
                            t[:], featT_d[h * 128:(h + 1) * 128, c0:c0 + 512])
                        pair.append(t)
                    ft_tiles[j] = pair
                    ft_next[0] += 1

            h_writes = [[] for _ in range(NQ)]

            def phase_a(q, rels=(0, 1)):
                """Build H table for bucket q, given relations, write to HBM."""
                for r in rels:
                    for ntl in range(BUCKETS[q] // 512):
                        j = loop_of[q, r, ntl]
                        ensure_ft(j + 2)
                        ft = ft_tiles.pop(j)
                        z1s = {}
                        for do_h in range(2):
                            z1p = psA.tile([128, 512], f32, tag="z1",
                                           space="PSUM", bufs=2)
                            for di_h in range(2):
                                nc.tensor.matmul(
                                    z1p[:],
                                    lhsT=W_rT_sb[r, di_h][:, do_h * 128:(do_h + 1) * 128],
                                    rhs=ft[di_h][:],
                                    start=(di_h == 0), stop=(di_h == 1))
                            # per-partition bias + relu + cast on the ACT unit
                            z = wp.tile([128, 512], bf16, tag=f"z1s{do_h}")
                            nc.scalar.activation(z[:], z1p[:], Relu,
                                                 bias=b_r_col_sb[r, do_h][:],
                                                 scale=1.0)
                            z1s[do_h] = z
                        hs = wp.tile([128, 4, D], bf16, tag="hs", bufs=2)
                        for cp2 in range(2):     # pairs of 128-node chunks
                            hp = psA.tile([128, 512], f32, tag="hp",
                                          space="PSUM", bufs=2)
                            for c4 in range(2):
                                sl = slice((cp2 * 2 + c4) * 128,
                                           (cp2 * 2 + c4 + 1) * 128)
                                dst_sl = slice(c4 * 256, (c4 + 1) * 256)
                                nc.tensor.matmul(hp[:, dst_sl],
                                                 lhsT=z1s[0][:, sl],
                                                 rhs=W_rT_sb[r, 0][:],
                                                 start=True, stop=False)
                                nc.tensor.matmul(hp[:, dst_sl],
                                                 lhsT=z1s[1][:, sl],
                                                 rhs=W_rT_sb[r, 1][:],
                                                 start=False, stop=True)
                            # bias lives on the free dim: DVE add (bf16 out),
                            # then ACT relu straight into the H staging tile
                            hpt = wp.tile([128, 512], bf16, tag="hpt", bufs=2)
                            nc.vector.tensor_tensor(out=hpt[:], in0=hp[:],
                                                    in1=b_r_rep2_sb[r][:],
                                                    op=Add)
                            nc.scalar.activation(hs[:, cp2 * 2:cp2 * 2 + 2, :],
                                                 hpt[:], Relu, bias=0.0,
                                                 scale=1.0)
                        row0 = r * BUCKETS[q] + ntl * 512
                        w = nc.sync.dma_start(
                            H_d[q][row0:row0 + 512, :].rearrange(
                                "(c p) d -> p c d", p=128),
                            hs[:])
                        h_writes[q].append(w.ins)

            cnt_reg = nc.gpsimd.alloc_register("cnt_reg")

            def gather_sub(i, q, t0, tb):
                """One dma_gather sub-group; returns (msup, sel)."""
                sel_t = wp.tile([128, SUB, SELW], fp8, tag="sel", bufs=3,
                                name=f"sel{i}")
                nc.gpsimd.dma_start(sel_t[:, 0:tb, :], sel_d[:, t0:t0 + tb, :])
                msup = wp.tile([128, SUB, D], bf16, tag="msup", bufs=4)
                nc.gpsimd.reg_load(cnt_reg, cnts_sb[:, i:i + 1])
                g_inst = nc.gpsimd.dma_gather(
                    out_ap=msup[:, 0:tb, :], in_ap=H_d[q][:],
                    idxs_ap=idxs_sb[:, t0 * 8:(t0 + tb) * 8],
                    num_idxs=tb * 128, num_idxs_reg=cnt_reg,
                    elem_size=D, single_packet=False)
                for w in h_writes[q]:
                    add_dep_helper(g_inst.ins, w,
                                   reason="gather waits on Htab writes")
                return msup, sel_t

            def seg_sum(q, b):
                """Weighted segment-sum for group (q, b): aggT += msup^T @ sel."""
                first = (q == 0)
                pb = {}
                for fh in range(2):
                    pb[fh, 0] = psB.tile([128, 512], f32, tag="pb512",
                                         space="PSUM", bufs=2, name=f"pb512_{fh}")
                    pb[fh, 1] = psB.tile([128, 128], f32, tag="pb128",
                                         space="PSUM", bufs=2, name=f"pb128_{fh}")
                sub_list = subs_of[q, b]
                n_tiles = sum(tb for _, _, tb in sub_list)
                done = 0
                for i, t0, tb in sub_list:
                    msup, sel_t = gather_sub(i, q, t0, tb)
                    for t in range(tb):
                        st = (done + t == 0)
                        sp = (done + t == n_tiles - 1)
                        for fh in range(2):
                            lhsT = msup[:, t, fh * 128:(fh + 1) * 128]
                            nc.tensor.matmul(pb[fh, 0][:], lhsT=lhsT,
                                             rhs=sel_t[:, t, 0:512],
                                             start=st, stop=sp)
                            nc.tensor.matmul(pb[fh, 1][:], lhsT=lhsT,
                                             rhs=sel_t[:, t, 512:SELW],
                                             start=st, stop=sp)
                    done += tb
                for fh in range(2):
                    if first:
                        nc.vector.tensor_copy(aggT_sb[b, fh][:, 0:512],
                                              pb[fh, 0][:])
                        nc.vector.tensor_copy(aggT_sb[b, fh][:, 512:SELW],
                                              pb[fh, 1][:])
                    else:
                        nc.vector.tensor_add(aggT_sb[b, fh][:, 0:512],
                                             aggT_sb[b, fh][:, 0:512], pb[fh, 0][:])
                        nc.vector.tensor_add(aggT_sb[b, fh][:, 512:SELW],
                                             aggT_sb[b, fh][:, 512:SELW], pb[fh, 1][:])

            def phase_c(b):
                """LSTM step + MLP for block b, transposed layout, batched."""
                n0 = b * BLK
                nn = min(BLK, NPC - n0)
                aggT = {}
                for fh in range(2):
                    t = wp.tile([128, SELW], bf16, tag=f"aggb{fh}", bufs=2)
                    nc.vector.tensor_copy(t[:], aggT_sb[b, fh][:])
                    aggT[fh] = t
                for ck, (cs, cw) in enumerate(((0, 512), (512, 128))):
                    if cs >= nn:
                        continue
                    cw_v = min(cw, nn - cs)    # valid columns this chunk
                    gate_sb = {}
                    for gi, gname, fn in ((0, "i", Sig), (1, "g", Tanh),
                                          (2, "o", Sig)):
                        for gh in range(2):
                            gp = psA.tile([128, 512], f32, tag="z1",
                                          space="PSUM", bufs=2)
                            gc = gi * 2 + gh
                            for h in range(2):
                                nc.tensor.matmul(
                                    gp[:, 0:cw],
                                    lhsT=CW["wih", h][:, gc * 128:(gc + 1) * 128],
                                    rhs=aggT[h][:, cs:cs + cw],
                                    start=(h == 0), stop=(h == 1))
                            a = wp.tile([128, 512], f32, tag=f"gact{gh}", bufs=2)
                            nc.scalar.activation(a[:, 0:cw], gp[:, 0:cw], fn,
                                                 bias=CW["bg", gc][:], scale=1.0)
                            gate_sb[gname, gh] = a
                    hnT = {}
                    for gh in range(2):
                        cc = wp.tile([128, 512], f32, tag=f"cc{gh}", bufs=2)
                        nc.vector.tensor_mul(cc[:, 0:cw], gate_sb["i", gh][:, 0:cw],
                                             gate_sb["g", gh][:, 0:cw])
                        nc.scalar.activation(cc[:, 0:cw], cc[:, 0:cw], Tanh,
                                             bias=0.0, scale=1.0)
                        hn = wp.tile([128, 512], bf16, tag=f"hn{gh}", bufs=2)
                        nc.vector.tensor_mul(hn[:, 0:cw], gate_sb["o", gh][:, 0:cw],
                                             cc[:, 0:cw])
                        hnT[gh] = hn
                    # MLP: x1 = relu(W1 hn + b1), x2 = relu(W2 x1 + b2)
                    x1p = psA.tile([128, 512], f32, tag="z1",
                                   space="PSUM", bufs=2)
                    for h in range(2):
                        nc.tensor.matmul(x1p[:, 0:cw], lhsT=CW["w1t", h][:],
                                         rhs=hnT[h][:, 0:cw],
                                         start=(h == 0), stop=(h == 1))
                    x1s = wp.tile([128, 512], bf16, tag="x1s", bufs=2)
                    nc.scalar.activation(x1s[:, 0:cw], x1p[:, 0:cw], Relu,
                                         bias=CW["b1"][:], scale=1.0)
                    x2p = psA.tile([128, 512], f32, tag="z1",
                                   space="PSUM", bufs=2)
                    nc.tensor.matmul(x2p[:, 0:cw], lhsT=CW["w2t"][:],
                                     rhs=x1s[:, 0:cw], start=True, stop=True)
                    x2s = wp.tile([128, 512], bf16, tag="x2s", bufs=2)
                    nc.scalar.activation(x2s[:, 0:cw], x2p[:, 0:cw], Relu,
                                         bias=CW["b2"][:], scale=1.0)
                    for oh in range(2):
                        op = psA.tile([128, 512], f32, tag="z1",
                                      space="PSUM", bufs=2)
                        nc.tensor.matmul(op[:, 0:cw],
                                         lhsT=CW["w3t"][:, oh * 128:(oh + 1) * 128],
                                         rhs=x2s[:, 0:cw], start=True, stop=True)
                        osb = wp.tile([128, 512], f32, tag=f"osb{oh}", bufs=1)
                        nc.vector.tensor_scalar_add(
                            osb[:, 0:cw], op[:, 0:cw], CW["b3", oh][:])
                        nc.sync.dma_start(
                            outT_d[oh * 128:(oh + 1) * 128,
                                   n0 + cs:n0 + cs + cw_v],
                            osb[:, 0:cw_v])

            # ---- schedule: interleave phase A (producer) with phase B ----
            phase_a(0)
            phase_a(1)
            seg_sum(0, 0)
            seg_sum(0, 1)
            phase_a(2)
            seg_sum(1, 0)
            seg_sum(1, 1)
            phase_a(3)
            seg_sum(2, 0)
            seg_sum(2, 1)
            phase_a(4)
            load_phase_c_weights()
            seg_sum(3, 0)
            seg_sum(3, 1)
            seg_sum(4, 0)
            phase_c(0)
            seg_sum(4, 1)
            phase_c(1)

    nc.compile()
    return nc


_CACHE = {}


def _get_nc(tiles_key):
    if tiles_key not in _CACHE:
        _CACHE[tiles_key] = _build([list(r) for r in tiles_key])
    return _CACHE[tiles_key]


# ----------------------------------------------------------------------------
# public entry
# ----------------------------------------------------------------------------

def kernel(**inputs) -> np.ndarray:
    src = np.asarray(inputs["src"], dtype=np.int32)
    dst = np.asarray(inputs["dst"], dtype=np.int32)
    rel = np.asarray(inputs["rel"], dtype=np.int32)

    com = _prep_weights(inputs)
    tiles_bq, idxs16_all, sel_all, cnts_all = _prep_edges(src, dst, rel)

    nc = _get_nc(tiles_bq)

    in_maps = []
    for c in range(NCORES):
        m = dict(com)
        m["idxs16"] = np.ascontiguousarray(idxs16_all[c])
        m["sel"] = np.ascontiguousarray(sel_all[c])
        m["cnts"] = np.ascontiguousarray(cnts_all[c])
        in_maps.append(m)

    res = bass_utils.run_bass_kernel_spmd(nc, in_maps, core_ids=list(range(NCORES)))

    out = np.empty((N_NODES, D_OUT), dtype=np.float32)
    for c in range(NCORES):
        out[c * NPC:(c + 1) * NPC, :] = res.results[c]["outT"].T
    return out


# revision 20
# speedup vs baseline: 1.1393x; 1.1088x over previous
"""Trainium2 Bass kernel for a relational GCN layer (message passing + LSTM + MLP).

Math (reference):
  S = feat[src]                               # [E, D] gather
  msgs[e] = edge_nn(S[e], W_rel[rel[e]])      # tied 2-layer relu MLP per relation
  agg = segment_sum(msgs, dst, N)             # [N, D]
  hn = LSTM-step(agg) from zero state         # [N, D]
  out = MLP(hn)                               # [N, D_OUT]

Key algebraic optimizations:
  1. msgs[e] depends only on (rel[e], src[e]): precompute the transformed node
     table H[r, s] = edge_nn(feat[s], W_rel[r]) once (dense GEMMs), then the
     edge phase is row-gather + weighted segment-sum.
  2. The SWDGE gather descriptor generation (~9 ns/row serialized on the
     GPSIMD engine) is the hard floor, so gathered rows are DEDUPLICATED per
     (dst-block of 625 nodes, src-bucket): each distinct (rel, src) is pulled
     once per block; edge multiplicity moves into a host-precomputed fp8
     count-selector matrix sel[slot, dst_local] streamed from HBM.
  3. The segment-sum matmul runs transposed: aggT[feat, dst] += msup^T @ sel
     (gathered tile stationary, 640-wide selector moving), so aggregates land
     feature-major and the LSTM/MLP phase needs no transposes and gets
     per-partition biases for free via the ACT unit.

Distribution: edges are sharded by DESTINATION node range across the 8 cores
(core c owns dst in [1250c, 1250(c+1))): every core computes complete
aggregates for its own 1250 nodes, no cross-core communication.

Pipeline: H is built in 5 src-buckets of growing size; the gather chain for
bucket q starts as soon as bucket q's H is in HBM.  Gathers run in sub-groups
of <=16 tiles; per-core valid counts are passed via a GPSIMD register
(num_idxs_reg) with trailing -1 index padding so cross-core padding costs no
descriptor-generation time.
"""

import math
import numpy as np
import ml_dtypes

import concourse.bacc as bacc
import concourse.bass as bass
import concourse.mybir as mybir
import concourse.tile as tile
from concourse import bass_utils
from concourse.tile import add_dep_helper

# ---- problem constants (hardcoded per spec) ----
N_NODES = 10000
N_EDGES = 320000
D = 256
D_OUT = 256
NUM_REL = 2
NCORES = 8
NPC = N_NODES // NCORES          # 1250 nodes per core
BLK = 625                        # dst-block size (2 blocks per core)
NBLK = 2
SELW = 640                       # sel matrix width (625 padded to 128-mult)
SUB = 16                         # max tiles per dma_gather sub-group
NPAD = 10240                     # node count padded to 20 tiles of 512
BUCKETS = [512, 1024, 2048, 3072, 3584]
BUCKET_BASE = [0, 512, 1536, 3584, 6656]
NQ = len(BUCKETS)

f32 = mybir.dt.float32
bf16 = mybir.dt.bfloat16
fp8 = mybir.dt.float8e4
u32 = mybir.dt.uint32
i16 = mybir.dt.int16

np_bf16 = ml_dtypes.bfloat16
np_fp8 = mybir.dt.np(fp8)


# ----------------------------------------------------------------------------
# host-side preprocessing
# ----------------------------------------------------------------------------

def _prep_edges(src, dst, rel):
    """Dedup edges per (core, dst-block, src-bucket); build gather index
    bands, fp8 count-selector matrices, and per-sub-group valid counts with a
    static tiling common to all cores.

    Slot p of a group lands at msup[p % 128, tile_ofs + p // 128, :]; its
    gather index goes to idxs16[p % 16, tile_ofs*8 + (p//2048)*128 + (p%2048)//16]
    (sub-groups of SUB=16 tiles are separate dma_gather calls over column
    slices of the band).
    """
    src = src.astype(np.int64)
    dst = dst.astype(np.int64)
    rel = rel.astype(np.int64)
    base = np.asarray(BUCKET_BASE, dtype=np.int64)
    sizes = np.asarray(BUCKETS, dtype=np.int64)
    q = np.searchsorted(base, src, side="right") - 1
    core = dst // NPC
    loc = dst % NPC
    blk = (loc >= BLK).astype(np.int64)
    dstloc = loc - blk * BLK
    rowid = rel * sizes[q] + (src - base[q])          # < 2*3584 = 7168
    gid = (core * NBLK + blk) * NQ + q                # 0..79
    key = gid * 8192 + rowid

    uk, inv = np.unique(key, return_inverse=True)
    gid_u = uk >> 13
    rowid_u = uk & 8191
    # position of each unique slot within its gid (uk is sorted by key)
    grp_first = np.searchsorted(uk >> 13, np.arange(NCORES * NBLK * NQ))
    pos_u = np.arange(len(uk)) - grp_first[gid_u]
    n_per_gid = np.bincount(gid_u, minlength=NCORES * NBLK * NQ)

    # static tiles per (blk, q) group: max over cores, in (q, blk) order
    n_cbq = n_per_gid.reshape(NCORES, NBLK, NQ)
    tiles_bq = [[int(math.ceil(n_cbq[:, b, qq].max() / 128))
                 for b in range(NBLK)] for qq in range(NQ)]
    grp_ofs = {}
    acc = 0
    subs = []            # static list of (q, b, tile_ofs, tb) sub-groups
    for qq in range(NQ):
        for b in range(NBLK):
            grp_ofs[qq, b] = acc
            t = tiles_bq[qq][b]
            s0 = 0
            while s0 < t:
                tb = min(SUB, t - s0)
                subs.append((qq, b, acc + s0, tb))
                s0 += tb
            acc += t
    nt = acc
    nsub = len(subs)

    idxs16_all = np.full((NCORES, 16, nt * 8), -1, dtype=np.int16)
    sel_i = np.zeros((NCORES, 128, nt, SELW), dtype=np.int16)
    cnts_all = np.zeros((NCORES, 1, nsub), dtype=np.uint32)

    # scatter unique-slot gather indices
    core_u = gid_u // (NBLK * NQ)
    b_u = (gid_u // NQ) % NBLK
    q_u = gid_u % NQ
    ofs_u = np.array([grp_ofs[qq, b] for qq in range(NQ) for b in range(NBLK)]
                     ).reshape(NQ, NBLK)[q_u, b_u]
    col = ofs_u * 8 + (pos_u // 2048) * 128 + (pos_u % 2048) // 16
    idxs16_all[core_u, pos_u % 16, col] = rowid_u.astype(np.int16)

    # per-edge multiplicity into sel (int accumulate, then fp8: exact <= 16)
    tile_e = ofs_u[inv] + pos_u[inv] // 128
    np.add.at(sel_i, (core_u[inv], pos_u[inv] % 128, tile_e, dstloc), 1)
    assert sel_i.max() <= 16, sel_i.max()
    sel_all = sel_i.astype(np_fp8)

    # per-sub-group valid counts; empty sub-groups get one sentinel row 0.
    # The first 4 sub-gathers (one per msup ring buffer) gather their full
    # padded count (pad indices 0) so every ring buffer starts fully written
    # with finite data; later sub-gathers may leave stale-but-finite tails.
    for c in range(NCORES):
        for i, (qq, b, t0, tb) in enumerate(subs):
            s0 = (t0 - grp_ofs[qq, b]) * 128
            n = int(n_cbq[c, b, qq])
            cnt = max(0, min(n - s0, tb * 128))
            if i < 4:
                band = idxs16_all[c, :, t0 * 8:(t0 + tb) * 8]
                band[band < 0] = 0
                cnt = tb * 128
            elif cnt == 0:
                idxs16_all[c, 0, t0 * 8] = 0    # sentinel: gather row 0
                cnt = 1
            cnts_all[c, 0, i] = cnt

    idxs16_all = np.tile(idxs16_all, (1, 8, 1))   # replicate to 128 partitions
    return tuple(tuple(r) for r in tiles_bq), idxs16_all, sel_all, cnts_all


def _prep_weights(inputs):
    feat = np.asarray(inputs["feat"], dtype=np.float32)
    W_rel = np.asarray(inputs["W_rel"], dtype=np.float32)
    b_rel = np.asarray(inputs["b_rel"], dtype=np.float32)
    W_ih = np.asarray(inputs["W_ih"], dtype=np.float32)
    b_ih = np.asarray(inputs["b_ih"], dtype=np.float32)
    b_hh = np.asarray(inputs["b_hh"], dtype=np.float32)
    W1 = np.asarray(inputs["W1"], dtype=np.float32)
    b1 = np.asarray(inputs["b1"], dtype=np.float32)
    W2 = np.asarray(inputs["W2"], dtype=np.float32)
    b2 = np.asarray(inputs["b2"], dtype=np.float32)
    W3 = np.asarray(inputs["W3"], dtype=np.float32)
    b3 = np.asarray(inputs["b3"], dtype=np.float32)

    featT = np.zeros((D, NPAD), dtype=np.float32)
    featT[:, :N_NODES] = feat.T
    keep = np.r_[0:256, 512:1024]  # i, g, o gate rows (f unused: c0 = 0)
    com = {
        "featT": featT.astype(np_bf16),
        "W_rT": np.ascontiguousarray(np.transpose(W_rel, (0, 2, 1))).astype(np_bf16),
        "b_r_col": np.ascontiguousarray(b_rel[:, :, None]),                  # f32
        "b_r_rep2": np.ascontiguousarray(np.broadcast_to(
            np.tile(b_rel, (1, 2))[:, None, :], (NUM_REL, 128, 2 * D))).copy(),
        "W_ihT": np.ascontiguousarray(W_ih[keep, :].T).astype(np_bf16),      # [256,768]
        "b_g_col": np.ascontiguousarray(
            (b_ih + b_hh)[keep].reshape(6, 128, 1)),                         # f32
        "W1T": np.ascontiguousarray(W1.T).astype(np_bf16),                   # [256,128]
        "b1_col": np.ascontiguousarray(b1[:, None]),
        "W2T": np.ascontiguousarray(W2.T).astype(np_bf16),                   # [128,128]
        "b2_col": np.ascontiguousarray(b2[:, None]),
        "W3T": np.ascontiguousarray(W3.T).astype(np_bf16),                   # [128,256]
        "b3_col": np.ascontiguousarray(b3.reshape(2, 128, 1)),
    }
    return com


# ----------------------------------------------------------------------------
# kernel builder
# ----------------------------------------------------------------------------

def _build(tiles_bq):
    Relu = mybir.ActivationFunctionType.Relu
    Sig = mybir.ActivationFunctionType.Sigmoid
    Tanh = mybir.ActivationFunctionType.Tanh
    Add = mybir.AluOpType.add
    Max = mybir.AluOpType.max

    grp_ofs = {}
    acc = 0
    subs = []
    for qq in range(NQ):
        for b in range(NBLK):
            grp_ofs[qq, b] = acc
            t = tiles_bq[qq][b]
            s0 = 0
            while s0 < t:
                tb = min(SUB, t - s0)
                subs.append((qq, b, acc + s0, tb))
                s0 += tb
            acc += t
    nt = acc
    nsub = len(subs)
    subs_of = {}
    for i, (qq, b, t0, tb) in enumerate(subs):
        subs_of.setdefault((qq, b), []).append((i, t0, tb))

    nc = bacc.Bacc("TRN2", target_bir_lowering=False, debug=False,
                   dynamic_dma_scratch_size=65536)

    featT_d = nc.dram_tensor("featT", [D, NPAD], bf16, kind="ExternalInput")
    W_rT_d = nc.dram_tensor("W_rT", [NUM_REL, D, D], bf16, kind="ExternalInput")
    b_r_col_d = nc.dram_tensor("b_r_col", [NUM_REL, D, 1], f32, kind="ExternalInput")
    b_r_rep2_d = nc.dram_tensor("b_r_rep2", [NUM_REL, 128, 2 * D], f32, kind="ExternalInput")
    W_ihT_d = nc.dram_tensor("W_ihT", [D, 768], bf16, kind="ExternalInput")
    b_g_col_d = nc.dram_tensor("b_g_col", [6, 128, 1], f32, kind="ExternalInput")
    W1T_d = nc.dram_tensor("W1T", [D, 128], bf16, kind="ExternalInput")
    b1_col_d = nc.dram_tensor("b1_col", [128, 1], f32, kind="ExternalInput")
    W2T_d = nc.dram_tensor("W2T", [128, 128], bf16, kind="ExternalInput")
    b2_col_d = nc.dram_tensor("b2_col", [128, 1], f32, kind="ExternalInput")
    W3T_d = nc.dram_tensor("W3T", [128, D_OUT], bf16, kind="ExternalInput")
    b3_col_d = nc.dram_tensor("b3_col", [2, 128, 1], f32, kind="ExternalInput")
    idxs_d = nc.dram_tensor("idxs16", [128, nt * 8], i16, kind="ExternalInput")
    sel_d = nc.dram_tensor("sel", [128, nt, SELW], fp8, kind="ExternalInput")
    cnts_d = nc.dram_tensor("cnts", [1, nsub], u32, kind="ExternalInput")

    outT_d = nc.dram_tensor("outT", [D_OUT, NPC], f32, kind="ExternalOutput")

    H_d = [nc.dram_tensor(f"Htab{g}", [NUM_REL * BUCKETS[g], D], bf16)
           for g in range(NQ)]

    with tile.TileContext(nc) as tc:
        with (
            tc.tile_pool(name="const", bufs=1) as cp,
            tc.tile_pool(name="work", bufs=3) as wp,
            tc.tile_pool(name="aggpool", bufs=1) as ap_pool,
            tc.tile_pool(name="psA", bufs=2, space="PSUM") as psA,
            tc.tile_pool(name="psB", bufs=2, space="PSUM") as psB,
        ):
            # ---- gather tables on the gpsimd queue (first: gathers need them) ----
            idxs_sb = cp.tile([128, nt * 8], i16, tag="idxs")
            nc.gpsimd.dma_start(idxs_sb[:], idxs_d[:, :])
            cnts_sb = cp.tile([1, nsub], u32, tag="cnts")
            nc.gpsimd.dma_start(cnts_sb[:], cnts_d[:, :])

            # phase-A weights on the scalar queue (short; sync stays free for
            # featT streaming)
            W_rT_sb = {}
            for r in range(NUM_REL):
                for h in range(2):
                    t = cp.tile([128, D], bf16, tag=f"wrt{r}{h}")
                    nc.gpsimd.dma_start(t[:], W_rT_d[r, h * 128:(h + 1) * 128, :])
                    W_rT_sb[r, h] = t
            b_r_col_sb = {}
            for r in range(NUM_REL):
                for h in range(2):
                    t = cp.tile([128, 1], f32, tag=f"brc{r}{h}")
                    nc.gpsimd.dma_start(t[:], b_r_col_d[r, h * 128:(h + 1) * 128, :])
                    b_r_col_sb[r, h] = t
            b_r_rep2_sb = {}
            for r in range(NUM_REL):
                t = cp.tile([128, 2 * D], f32, tag=f"brr{r}")
                nc.gpsimd.dma_start(t[:], b_r_rep2_d[r, :, :])
                b_r_rep2_sb[r] = t

            # zero the msup gather ring once: the trimmed gathers leave tail
            # slots (and whole tail tiles of short sub-groups) unwritten, and
            # 0 * garbage is only 0 when the stale bits are finite.  After
            # this, stale data is always old H rows.
            for k in range(4):
                t = wp.tile([128, SUB, D], bf16, tag="msup", bufs=4,
                            name=f"msup_init{k}")
                nc.vector.memset(t[:], 0.0)

            # SBUF aggregate accumulators, transposed: [feat_half, 640 dst]
            aggT_sb = {}
            for b in range(NBLK):
                for fh in range(2):
                    aggT_sb[b, fh] = ap_pool.tile(
                        [128, SELW], f32, tag=f"agg{b}{fh}", name=f"agg{b}{fh}")

            # phase-C weights: emitted onto the sync queue after all phase-A
            # featT traffic (see schedule below); dict filled lazily
            CW = {}

            def load_phase_c_weights():
                for h in range(2):
                    t = cp.tile([128, 768], bf16, tag=f"wih{h}", name=f"wih{h}")
                    nc.sync.dma_start(t[:], W_ihT_d[h * 128:(h + 1) * 128, :])
                    CW["wih", h] = t
                for g in range(6):
                    t = cp.tile([128, 1], f32, tag=f"bg{g}", name=f"bg{g}")
                    nc.sync.dma_start(t[:], b_g_col_d[g, :, :])
                    CW["bg", g] = t
                for h in range(2):
                    t = cp.tile([128, 128], bf16, tag=f"w1t{h}", name=f"w1t{h}")
                    nc.sync.dma_start(t[:], W1T_d[h * 128:(h + 1) * 128, :])
                    CW["w1t", h] = t
                t = cp.tile([128, 1], f32, tag="b1", name="b1c")
                nc.sync.dma_start(t[:], b1_col_d[:, :])
                CW["b1"] = t
                t = cp.tile([128, 128], bf16, tag="w2t", name="w2t")
                nc.sync.dma_start(t[:], W2T_d[:, :])
                CW["w2t"] = t
                t = cp.tile([128, 1], f32, tag="b2", name="b2c")
                nc.sync.dma_start(t[:], b2_col_d[:, :])
                CW["b2"] = t
                t = cp.tile([128, D_OUT], bf16, tag="w3t", name="w3t")
                nc.sync.dma_start(t[:], W3T_d[:, :])
                CW["w3t"] = t
                for h in range(2):
                    t = cp.tile([128, 1], f32, tag=f"b3{h}", name=f"b3c{h}")
                    nc.sync.dma_start(t[:], b3_col_d[h, :, :])
                    CW["b3", h] = t

            # ---- phase A with featT prefetch 2 loops ahead (cross-quarter) ----
            all_loops = [(q, r, ntl) for q in range(NQ) for r in range(NUM_REL)
                         for ntl in range(BUCKETS[q] // 512)]
            loop_of = {}
            for j, (q, r, ntl) in enumerate(all_loops):
                loop_of[q, r, ntl] = j
            ft_tiles = {}
            ft_next = [0]

            def ensure_ft(upto):
                while ft_next[0] <= min(upto, len(all_loops) - 1):
                    j = ft_next[0]
                    q, r, ntl = all_loops[j]
                    c0 = BUCKET_BASE[q] + ntl * 512
                    pair = []
                    for h in range(2):
                        t = wp.tile([128, 512], bf16, tag=f"ft{h}", bufs=3,
                                    name=f"ft{h}_{j}")
                        nc.sync.dma_start(
                            t[:], featT_d[h * 128:(h + 1) * 128, c0:c0 + 512])
                        pair.append(t)
                    ft_tiles[j] = pair
                    ft_next[0] += 1

            h_writes = [[] for _ in range(NQ)]

            def phase_a(q, rels=(0, 1)):
                """Build H table for bucket q, given relations, write to HBM."""
                for r in rels:
                    for ntl in range(BUCKETS[q] // 512):
                        j = loop_of[q, r, ntl]
                        ensure_ft(j + 2)
                        ft = ft_tiles.pop(j)
                        z1s = {}
                        for do_h in range(2):
                            z1p = psA.tile([128, 512], f32, tag="z1",
                                           space="PSUM", bufs=2)
                            for di_h in range(2):
                                nc.tensor.matmul(
                                    z1p[:],
                                    lhsT=W_rT_sb[r, di_h][:, do_h * 128:(do_h + 1) * 128],
                                    rhs=ft[di_h][:],
                                    start=(di_h == 0), stop=(di_h == 1))
                            # per-partition bias + relu + cast on the ACT unit
                            z = wp.tile([128, 512], bf16, tag=f"z1s{do_h}")
                            nc.scalar.activation(z[:], z1p[:], Relu,
                                                 bias=b_r_col_sb[r, do_h][:],
                                                 scale=1.0)
                            z1s[do_h] = z
                        hs = wp.tile([128, 4, D], bf16, tag="hs", bufs=2)
                        for cp2 in range(2):     # pairs of 128-node chunks
                            hp = psA.tile([128, 512], f32, tag="hp",
                                          space="PSUM", bufs=2)
                            for c4 in range(2):
                                sl = slice((cp2 * 2 + c4) * 128,
                                           (cp2 * 2 + c4 + 1) * 128)
                                dst_sl = slice(c4 * 256, (c4 + 1) * 256)
                                nc.tensor.matmul(hp[:, dst_sl],
                                                 lhsT=z1s[0][:, sl],
                                                 rhs=W_rT_sb[r, 0][:],
                                                 start=True, stop=False)
                                nc.tensor.matmul(hp[:, dst_sl],
                                                 lhsT=z1s[1][:, sl],
                                                 rhs=W_rT_sb[r, 1][:],
                                                 start=False, stop=True)
                            # bias lives on the free dim: DVE add (bf16 out),
                            # then ACT relu straight into the H staging tile
                            hpt = wp.tile([128, 512], bf16, tag="hpt", bufs=2)
                            nc.vector.tensor_tensor(out=hpt[:], in0=hp[:],
                                                    in1=b_r_rep2_sb[r][:],
                                                    op=Add)
                            nc.scalar.activation(hs[:, cp2 * 2:cp2 * 2 + 2, :],
                                                 hpt[:], Relu, bias=0.0,
                                                 scale=1.0)
                        row0 = r * BUCKETS[q] + ntl * 512
                        w = nc.sync.dma_start(
                            H_d[q][row0:row0 + 512, :].rearrange(
                                "(c p) d -> p c d", p=128),
                            hs[:])
                        h_writes[q].append(w.ins)

            cnt_reg = nc.gpsimd.alloc_register("cnt_reg")

            def gather_sub(i, q, t0, tb):
                """One dma_gather sub-group; returns (msup, sel)."""
                sel_t = wp.tile([128, SUB, SELW], fp8, tag="sel", bufs=3,
                                name=f"sel{i}")
                nc.gpsimd.dma_start(sel_t[:, 0:tb, :], sel_d[:, t0:t0 + tb, :])
                msup = wp.tile([128, SUB, D], bf16, tag="msup", bufs=4)
                nc.gpsimd.reg_load(cnt_reg, cnts_sb[:, i:i + 1])
                g_inst = nc.gpsimd.dma_gather(
                    out_ap=msup[:, 0:tb, :], in_ap=H_d[q][:],
                    idxs_ap=idxs_sb[:, t0 * 8:(t0 + tb) * 8],
                    num_idxs=tb * 128, num_idxs_reg=cnt_reg,
                    elem_size=D, single_packet=False)
                for w in h_writes[q]:
                    add_dep_helper(g_inst.ins, w,
                                   reason="gather waits on Htab writes")
                return msup, sel_t

            def seg_sum(q, b):
                """Weighted segment-sum for group (q, b): aggT += msup^T @ sel."""
                first = (q == 0)
                pb = {}
                for fh in range(2):
                    pb[fh, 0] = psB.tile([128, 512], f32, tag="pb512",
                                         space="PSUM", bufs=2, name=f"pb512_{fh}")
                    pb[fh, 1] = psB.tile([128, 128], f32, tag="pb128",
                                         space="PSUM", bufs=2, name=f"pb128_{fh}")
                sub_list = subs_of[q, b]
                n_tiles = sum(tb for _, _, tb in sub_list)
                done = 0
                for i, t0, tb in sub_list:
                    msup, sel_t = gather_sub(i, q, t0, tb)
                    for t in range(tb):
                        st = (done + t == 0)
                        sp = (done + t == n_tiles - 1)
                        for fh in range(2):
                            lhsT = msup[:, t, fh * 128:(fh + 1) * 128]
                            nc.tensor.matmul(pb[fh, 0][:], lhsT=lhsT,
                                             rhs=sel_t[:, t, 0:512],
                                             start=st, stop=sp)
                            nc.tensor.matmul(pb[fh, 1][:], lhsT=lhsT,
                                             rhs=sel_t[:, t, 512:SELW],
                                             start=st, stop=sp)
                    done += tb
                for fh in range(2):
                    if first:
                        nc.vector.tensor_copy(aggT_sb[b, fh][:, 0:512],
                                              pb[fh, 0][:])
                        nc.vector.tensor_copy(aggT_sb[b, fh][:, 512:SELW],
                                              pb[fh, 1][:])
                    else:
                        nc.vector.tensor_add(aggT_sb[b, fh][:, 0:512],
                                             aggT_sb[b, fh][:, 0:512], pb[fh, 0][:])
                        nc.vector.tensor_add(aggT_sb[b, fh][:, 512:SELW],
                                             aggT_sb[b, fh][:, 512:SELW], pb[fh, 1][:])

            def phase_c(b):
                """LSTM step + MLP for block b, transposed layout, batched."""
                n0 = b * BLK
                nn = min(BLK, NPC - n0)
                aggT = {}
                for fh in range(2):
                    t = wp.tile([128, SELW], bf16, tag=f"aggb{fh}", bufs=2)
                    nc.vector.tensor_copy(t[:], aggT_sb[b, fh][:])
                    aggT[fh] = t
                for ck, (cs, cw) in enumerate(((0, 512), (512, 128))):
                    if cs >= nn:
                        continue
                    cw_v = min(cw, nn - cs)    # valid columns this chunk
                    gate_sb = {}
                    for gi, gname, fn in ((0, "i", Sig), (1, "g", Tanh),
                                          (2, "o", Sig)):
                        for gh in range(2):
                            gp = psA.tile([128, 512], f32, tag="z1",
                                          space="PSUM", bufs=2)
                            gc = gi * 2 + gh
                            for h in range(2):
                                nc.tensor.matmul(
                                    gp[:, 0:cw],
                                    lhsT=CW["wih", h][:, gc * 128:(gc + 1) * 128],
                                    rhs=aggT[h][:, cs:cs + cw],
                                    start=(h == 0), stop=(h == 1))
                            a = wp.tile([128, 512], f32, tag=f"gact{gh}", bufs=2)
                            nc.scalar.activation(a[:, 0:cw], gp[:, 0:cw], fn,
                                                 bias=CW["bg", gc][:], scale=1.0)
                            gate_sb[gname, gh] = a
                    hnT = {}
                    for gh in range(2):
                        cc = wp.tile([128, 512], f32, tag=f"cc{gh}", bufs=2)
                        nc.vector.tensor_mul(cc[:, 0:cw], gate_sb["i", gh][:, 0:cw],
                                             gate_sb["g", gh][:, 0:cw])
                        nc.scalar.activation(cc[:, 0:cw], cc[:, 0:cw], Tanh,
                                             bias=0.0, scale=1.0)
                        hn = wp.tile([128, 512], bf16, tag=f"hn{gh}", bufs=2)
                        nc.vector.tensor_mul(hn[:, 0:cw], gate_sb["o", gh][:, 0:cw],
                                             cc[:, 0:cw])
                        hnT[gh] = hn
                    # MLP: x1 = relu(W1 hn + b1), x2 = relu(W2 x1 + b2)
                    x1p = psA.tile([128, 512], f32, tag="z1",
                                   space="PSUM", bufs=2)
                    for h in range(2):
                        nc.tensor.matmul(x1p[:, 0:cw], lhsT=CW["w1t", h][:],
                                         rhs=hnT[h][:, 0:cw],
                                         start=(h == 0), stop=(h == 1))
                    x1s = wp.tile([128, 512], bf16, tag="x1s", bufs=2)
                    nc.scalar.activation(x1s[:, 0:cw], x1p[:, 0:cw], Relu,
                                         bias=CW["b1"][:], scale=1.0)
                    x2p = psA.tile([128, 512], f32, tag="z1",
                                   space="PSUM", bufs=2)
                    nc.tensor.matmul(x2p[:, 0:cw], lhsT=CW["w2t"][:],
                                     rhs=x1s[:, 0:cw], start=True, stop=True)
                    x2s = wp.tile([128, 512], bf16, tag="x2s", bufs=2)
                    nc.scalar.activation(x2s[:, 0:cw], x2p[:, 0:cw], Relu,
                                         bias=CW["b2"][:], scale=1.0)
                    for oh in range(2):
                        op = psA.tile([128, 512], f32, tag="z1",
                                      space="PSUM", bufs=2)
                        nc.tensor.matmul(op[:, 0:cw],
                                         lhsT=CW["w3t"][:, oh * 128:(oh + 1) * 128],
                                         rhs=x2s[:, 0:cw], start=True, stop=True)
                        osb = wp.tile([128, 512], f32, tag=f"osb{oh}", bufs=1)
                        nc.vector.tensor_scalar_add(
                            osb[:, 0:cw], op[:, 0:cw], CW["b3", oh][:])
                        nc.sync.dma_start(
                            outT_d[oh * 128:(oh + 1) * 128,
                                   n0 + cs:n0 + cs + cw_v],
                            osb[:, 0:cw_v])

            # ---- schedule: interleave phase A (producer) with phase B at
            # per-relation granularity so the in-order PE stream never traps
            # the H-table producer behind long consumer batches ----
            phase_a(0)
            phase_a(1)
            seg_sum(0, 0)
            phase_a(2, rels=(0,))
            seg_sum(0, 1)
            phase_a(2, rels=(1,))
            seg_sum(1, 0)
            phase_a(3, rels=(0,))
            seg_sum(1, 1)
            phase_a(3, rels=(1,))
            seg_sum(2, 0)
            phase_a(4, rels=(0,))
            seg_sum(2, 1)
            phase_a(4, rels=(1,))
            load_phase_c_weights()
            seg_sum(3, 0)
            seg_sum(3, 1)
            seg_sum(4, 0)
            phase_c(0)
            seg_sum(4, 1)
            phase_c(1)

    nc.compile()
    return nc


_CACHE = {}


def _get_nc(tiles_key):
    if tiles_key not in _CACHE:
        _CACHE[tiles_key] = _build([list(r) for r in tiles_key])
    return _CACHE[tiles_key]


# ----------------------------------------------------------------------------
# public entry
# ----------------------------------------------------------------------------

def kernel(**inputs) -> np.ndarray:
    src = np.asarray(inputs["src"], dtype=np.int32)
    dst = np.asarray(inputs["dst"], dtype=np.int32)
    rel = np.asarray(inputs["rel"], dtype=np.int32)

    com = _prep_weights(inputs)
    tiles_bq, idxs16_all, sel_all, cnts_all = _prep_edges(src, dst, rel)

    nc = _get_nc(tiles_bq)

    in_maps = []
    for c in range(NCORES):
        m = dict(com)
        m["idxs16"] = np.ascontiguousarray(idxs16_all[c])
        m["sel"] = np.ascontiguousarray(sel_all[c])
        m["cnts"] = np.ascontiguousarray(cnts_all[c])
        in_maps.append(m)

    res = bass_utils.run_bass_kernel_spmd(nc, in_maps, core_ids=list(range(NCORES)))

    out = np.empty((N_NODES, D_OUT), dtype=np.float32)
    for c in range(NCORES):
        out[c * NPC:(c + 1) * NPC, :] = res.results[c]["outT"].T
    return out


# revision 21
# speedup vs baseline: 1.1583x; 1.0166x over previous
"""Trainium2 Bass kernel for a relational GCN layer (message passing + LSTM + MLP).

Math (reference):
  S = feat[src]                               # [E, D] gather
  msgs[e] = edge_nn(S[e], W_rel[rel[e]])      # tied 2-layer relu MLP per relation
  agg = segment_sum(msgs, dst, N)             # [N, D]
  hn = LSTM-step(agg) from zero state         # [N, D]
  out = MLP(hn)                               # [N, D_OUT]

Key algebraic optimizations:
  1. msgs[e] depends only on (rel[e], src[e]): precompute the transformed node
     table H[r, s] = edge_nn(feat[s], W_rel[r]) once (dense GEMMs), then the
     edge phase is row-gather + weighted segment-sum.
  2. The SWDGE gather descriptor generation (~9 ns/row serialized on the
     GPSIMD engine) is the hard floor, so gathered rows are DEDUPLICATED per
     (dst-block of 625 nodes, src-bucket): each distinct (rel, src) is pulled
     once per block; edge multiplicity moves into a host-precomputed fp8
     count-selector matrix sel[slot, dst_local] streamed from HBM.
  3. The segment-sum matmul runs transposed: aggT[feat, dst] += msup^T @ sel
     (gathered tile stationary, 640-wide selector moving), so aggregates land
     feature-major and the LSTM/MLP phase needs no transposes and gets
     per-partition biases for free via the ACT unit.

Distribution: edges are sharded by DESTINATION node range across the 8 cores
(core c owns dst in [1250c, 1250(c+1))): every core computes complete
aggregates for its own 1250 nodes, no cross-core communication.

Pipeline: H is built in 5 src-buckets of growing size; the gather chain for
bucket q starts as soon as bucket q's H is in HBM.  Gathers run in sub-groups
of <=16 tiles; per-core valid counts are passed via a GPSIMD register
(num_idxs_reg) with trailing -1 index padding so cross-core padding costs no
descriptor-generation time.
"""

import math
import numpy as np
import ml_dtypes

import concourse.bacc as bacc
import concourse.bass as bass
import concourse.mybir as mybir
import concourse.tile as tile
from concourse import bass_utils
from concourse.tile import add_dep_helper

# ---- problem constants (hardcoded per spec) ----
N_NODES = 10000
N_EDGES = 320000
D = 256
D_OUT = 256
NUM_REL = 2
NCORES = 8
NPC = N_NODES // NCORES          # 1250 nodes per core
BLK = 625                        # dst-block size (2 blocks per core)
NBLK = 2
SELW = 640                       # sel matrix width (625 padded to 128-mult)
SUB = 16                         # max tiles per dma_gather sub-group
NPAD = 10240                     # node count padded to 20 tiles of 512
BUCKETS = [512, 1024, 2048, 3072, 3584]
BUCKET_BASE = [0, 512, 1536, 3584, 6656]
NQ = len(BUCKETS)

f32 = mybir.dt.float32
bf16 = mybir.dt.bfloat16
fp8 = mybir.dt.float8e4
u32 = mybir.dt.uint32
i16 = mybir.dt.int16

np_bf16 = ml_dtypes.bfloat16
np_fp8 = mybir.dt.np(fp8)


# ----------------------------------------------------------------------------
# host-side preprocessing
# ----------------------------------------------------------------------------

def _prep_edges(src, dst, rel):
    """Dedup edges per (core, dst-block, src-bucket); build gather index
    bands, fp8 count-selector matrices, and per-sub-group valid counts with a
    static tiling common to all cores.

    Slot p of a group lands at msup[p % 128, tile_ofs + p // 128, :]; its
    gather index goes to idxs16[p % 16, tile_ofs*8 + (p//2048)*128 + (p%2048)//16]
    (sub-groups of SUB=16 tiles are separate dma_gather calls over column
    slices of the band).
    """
    src = src.astype(np.int64)
    dst = dst.astype(np.int64)
    rel = rel.astype(np.int64)
    base = np.asarray(BUCKET_BASE, dtype=np.int64)
    sizes = np.asarray(BUCKETS, dtype=np.int64)
    q = np.searchsorted(base, src, side="right") - 1
    core = dst // NPC
    loc = dst % NPC
    blk = (loc >= BLK).astype(np.int64)
    dstloc = loc - blk * BLK
    # logical H row, then permuted to the partition-major physical layout
    # (phase A writes H as [128, R/128, 256]: row r lives at physical index
    # (r%128)*(R/128) + r//128, giving contiguous per-partition HBM writes)
    rowl = rel * sizes[q] + (src - base[q])
    rchunks = 2 * sizes[q] // 128
    rowid = (rowl % 128) * rchunks + rowl // 128      # < 2*3584 = 7168
    gid = (core * NBLK + blk) * NQ + q                # 0..79
    key = gid * 8192 + rowid

    uk, inv = np.unique(key, return_inverse=True)
    gid_u = uk >> 13
    rowid_u = uk & 8191
    # position of each unique slot within its gid (uk is sorted by key)
    grp_first = np.searchsorted(uk >> 13, np.arange(NCORES * NBLK * NQ))
    pos_u = np.arange(len(uk)) - grp_first[gid_u]
    n_per_gid = np.bincount(gid_u, minlength=NCORES * NBLK * NQ)

    # static tiles per (blk, q) group: max over cores, in (q, blk) order
    n_cbq = n_per_gid.reshape(NCORES, NBLK, NQ)
    tiles_bq = [[int(math.ceil(n_cbq[:, b, qq].max() / 128))
                 for b in range(NBLK)] for qq in range(NQ)]
    grp_ofs = {}
    acc = 0
    subs = []            # static list of (q, b, tile_ofs, tb) sub-groups
    for qq in range(NQ):
        for b in range(NBLK):
            grp_ofs[qq, b] = acc
            t = tiles_bq[qq][b]
            s0 = 0
            while s0 < t:
                tb = min(SUB, t - s0)
                subs.append((qq, b, acc + s0, tb))
                s0 += tb
            acc += t
    nt = acc
    nsub = len(subs)

    idxs16_all = np.full((NCORES, 16, nt * 8), -1, dtype=np.int16)
    sel_i = np.zeros((NCORES, 128, nt, SELW), dtype=np.int16)
    cnts_all = np.zeros((NCORES, 1, nsub), dtype=np.uint32)

    # scatter unique-slot gather indices
    core_u = gid_u // (NBLK * NQ)
    b_u = (gid_u // NQ) % NBLK
    q_u = gid_u % NQ
    ofs_u = np.array([grp_ofs[qq, b] for qq in range(NQ) for b in range(NBLK)]
                     ).reshape(NQ, NBLK)[q_u, b_u]
    col = ofs_u * 8 + (pos_u // 2048) * 128 + (pos_u % 2048) // 16
    idxs16_all[core_u, pos_u % 16, col] = rowid_u.astype(np.int16)

    # per-edge multiplicity into sel (int accumulate, then fp8: exact <= 16)
    tile_e = ofs_u[inv] + pos_u[inv] // 128
    np.add.at(sel_i, (core_u[inv], pos_u[inv] % 128, tile_e, dstloc), 1)
    assert sel_i.max() <= 16, sel_i.max()
    sel_all = sel_i.astype(np_fp8)

    # per-sub-group valid counts; empty sub-groups get one sentinel row 0.
    # The first 4 sub-gathers (one per msup ring buffer) gather their full
    # padded count (pad indices 0) so every ring buffer starts fully written
    # with finite data; later sub-gathers may leave stale-but-finite tails.
    for c in range(NCORES):
        for i, (qq, b, t0, tb) in enumerate(subs):
            s0 = (t0 - grp_ofs[qq, b]) * 128
            n = int(n_cbq[c, b, qq])
            cnt = max(0, min(n - s0, tb * 128))
            if i < 4:
                band = idxs16_all[c, :, t0 * 8:(t0 + tb) * 8]
                band[band < 0] = 0
                cnt = tb * 128
            elif cnt == 0:
                idxs16_all[c, 0, t0 * 8] = 0    # sentinel: gather row 0
                cnt = 1
            cnts_all[c, 0, i] = cnt

    idxs16_all = np.tile(idxs16_all, (1, 8, 1))   # replicate to 128 partitions
    return tuple(tuple(r) for r in tiles_bq), idxs16_all, sel_all, cnts_all


def _prep_weights(inputs):
    feat = np.asarray(inputs["feat"], dtype=np.float32)
    W_rel = np.asarray(inputs["W_rel"], dtype=np.float32)
    b_rel = np.asarray(inputs["b_rel"], dtype=np.float32)
    W_ih = np.asarray(inputs["W_ih"], dtype=np.float32)
    b_ih = np.asarray(inputs["b_ih"], dtype=np.float32)
    b_hh = np.asarray(inputs["b_hh"], dtype=np.float32)
    W1 = np.asarray(inputs["W1"], dtype=np.float32)
    b1 = np.asarray(inputs["b1"], dtype=np.float32)
    W2 = np.asarray(inputs["W2"], dtype=np.float32)
    b2 = np.asarray(inputs["b2"], dtype=np.float32)
    W3 = np.asarray(inputs["W3"], dtype=np.float32)
    b3 = np.asarray(inputs["b3"], dtype=np.float32)

    featT = np.zeros((D, NPAD), dtype=np.float32)
    featT[:, :N_NODES] = feat.T
    keep = np.r_[0:256, 512:1024]  # i, g, o gate rows (f unused: c0 = 0)
    com = {
        "featT": featT.astype(np_bf16),
        "W_rT": np.ascontiguousarray(np.transpose(W_rel, (0, 2, 1))).astype(np_bf16),
        "b_r_col": np.ascontiguousarray(b_rel[:, :, None]),                  # f32
        "b_r_rep2": np.ascontiguousarray(np.broadcast_to(
            np.tile(b_rel, (1, 2))[:, None, :], (NUM_REL, 128, 2 * D))).copy(),
        "W_ihT": np.ascontiguousarray(W_ih[keep, :].T).astype(np_bf16),      # [256,768]
        "b_g_col": np.ascontiguousarray(
            (b_ih + b_hh)[keep].reshape(6, 128, 1)),                         # f32
        "W1T": np.ascontiguousarray(W1.T).astype(np_bf16),                   # [256,128]
        "b1_col": np.ascontiguousarray(b1[:, None]),
        "W2T": np.ascontiguousarray(W2.T).astype(np_bf16),                   # [128,128]
        "b2_col": np.ascontiguousarray(b2[:, None]),
        "W3T": np.ascontiguousarray(W3.T).astype(np_bf16),                   # [128,256]
        "b3_col": np.ascontiguousarray(b3.reshape(2, 128, 1)),
    }
    return com


# ----------------------------------------------------------------------------
# kernel builder
# ----------------------------------------------------------------------------

def _build(tiles_bq):
    Relu = mybir.ActivationFunctionType.Relu
    Sig = mybir.ActivationFunctionType.Sigmoid
    Tanh = mybir.ActivationFunctionType.Tanh
    Add = mybir.AluOpType.add
    Max = mybir.AluOpType.max

    grp_ofs = {}
    acc = 0
    subs = []
    for qq in range(NQ):
        for b in range(NBLK):
            grp_ofs[qq, b] = acc
            t = tiles_bq[qq][b]
            s0 = 0
            while s0 < t:
                tb = min(SUB, t - s0)
                subs.append((qq, b, acc + s0, tb))
                s0 += tb
            acc += t
    nt = acc
    nsub = len(subs)
    subs_of = {}
    for i, (qq, b, t0, tb) in enumerate(subs):
        subs_of.setdefault((qq, b), []).append((i, t0, tb))

    nc = bacc.Bacc("TRN2", target_bir_lowering=False, debug=False,
                   dynamic_dma_scratch_size=65536)

    featT_d = nc.dram_tensor("featT", [D, NPAD], bf16, kind="ExternalInput")
    W_rT_d = nc.dram_tensor("W_rT", [NUM_REL, D, D], bf16, kind="ExternalInput")
    b_r_col_d = nc.dram_tensor("b_r_col", [NUM_REL, D, 1], f32, kind="ExternalInput")
    b_r_rep2_d = nc.dram_tensor("b_r_rep2", [NUM_REL, 128, 2 * D], f32, kind="ExternalInput")
    W_ihT_d = nc.dram_tensor("W_ihT", [D, 768], bf16, kind="ExternalInput")
    b_g_col_d = nc.dram_tensor("b_g_col", [6, 128, 1], f32, kind="ExternalInput")
    W1T_d = nc.dram_tensor("W1T", [D, 128], bf16, kind="ExternalInput")
    b1_col_d = nc.dram_tensor("b1_col", [128, 1], f32, kind="ExternalInput")
    W2T_d = nc.dram_tensor("W2T", [128, 128], bf16, kind="ExternalInput")
    b2_col_d = nc.dram_tensor("b2_col", [128, 1], f32, kind="ExternalInput")
    W3T_d = nc.dram_tensor("W3T", [128, D_OUT], bf16, kind="ExternalInput")
    b3_col_d = nc.dram_tensor("b3_col", [2, 128, 1], f32, kind="ExternalInput")
    idxs_d = nc.dram_tensor("idxs16", [128, nt * 8], i16, kind="ExternalInput")
    sel_d = nc.dram_tensor("sel", [128, nt, SELW], fp8, kind="ExternalInput")
    cnts_d = nc.dram_tensor("cnts", [1, nsub], u32, kind="ExternalInput")

    outT_d = nc.dram_tensor("outT", [D_OUT, NPC], f32, kind="ExternalOutput")

    H_d = [nc.dram_tensor(f"Htab{g}", [128, NUM_REL * BUCKETS[g] // 128, D],
                          bf16) for g in range(NQ)]

    with tile.TileContext(nc) as tc:
        with (
            tc.tile_pool(name="const", bufs=1) as cp,
            tc.tile_pool(name="work", bufs=3) as wp,
            tc.tile_pool(name="aggpool", bufs=1) as ap_pool,
            tc.tile_pool(name="psA", bufs=2, space="PSUM") as psA,
            tc.tile_pool(name="psB", bufs=2, space="PSUM") as psB,
        ):
            # ---- gather tables on the gpsimd queue (first: gathers need them) ----
            idxs_sb = cp.tile([128, nt * 8], i16, tag="idxs")
            nc.gpsimd.dma_start(idxs_sb[:], idxs_d[:, :])
            cnts_sb = cp.tile([1, nsub], u32, tag="cnts")
            nc.gpsimd.dma_start(cnts_sb[:], cnts_d[:, :])

            # phase-A weights on the scalar queue (short; sync stays free for
            # featT streaming)
            W_rT_sb = {}
            for r in range(NUM_REL):
                for h in range(2):
                    t = cp.tile([128, D], bf16, tag=f"wrt{r}{h}")
                    nc.gpsimd.dma_start(t[:], W_rT_d[r, h * 128:(h + 1) * 128, :])
                    W_rT_sb[r, h] = t
            b_r_col_sb = {}
            for r in range(NUM_REL):
                for h in range(2):
                    t = cp.tile([128, 1], f32, tag=f"brc{r}{h}")
                    nc.gpsimd.dma_start(t[:], b_r_col_d[r, h * 128:(h + 1) * 128, :])
                    b_r_col_sb[r, h] = t
            b_r_rep2_sb = {}
            for r in range(NUM_REL):
                t = cp.tile([128, 2 * D], f32, tag=f"brr{r}")
                nc.gpsimd.dma_start(t[:], b_r_rep2_d[r, :, :])
                b_r_rep2_sb[r] = t

            # zero the msup gather ring once: the trimmed gathers leave tail
            # slots (and whole tail tiles of short sub-groups) unwritten, and
            # 0 * garbage is only 0 when the stale bits are finite.  After
            # this, stale data is always old H rows.
            for k in range(4):
                t = wp.tile([128, SUB, D], bf16, tag="msup", bufs=4,
                            name=f"msup_init{k}")
                nc.vector.memset(t[:], 0.0)

            # SBUF aggregate accumulators, transposed: [feat_half, 640 dst]
            aggT_sb = {}
            for b in range(NBLK):
                for fh in range(2):
                    aggT_sb[b, fh] = ap_pool.tile(
                        [128, SELW], f32, tag=f"agg{b}{fh}", name=f"agg{b}{fh}")

            # phase-C weights: emitted onto the sync queue after all phase-A
            # featT traffic (see schedule below); dict filled lazily
            CW = {}

            def load_phase_c_weights():
                for h in range(2):
                    t = cp.tile([128, 768], bf16, tag=f"wih{h}", name=f"wih{h}")
                    nc.sync.dma_start(t[:], W_ihT_d[h * 128:(h + 1) * 128, :])
                    CW["wih", h] = t
                for g in range(6):
                    t = cp.tile([128, 1], f32, tag=f"bg{g}", name=f"bg{g}")
                    nc.sync.dma_start(t[:], b_g_col_d[g, :, :])
                    CW["bg", g] = t
                for h in range(2):
                    t = cp.tile([128, 128], bf16, tag=f"w1t{h}", name=f"w1t{h}")
                    nc.sync.dma_start(t[:], W1T_d[h * 128:(h + 1) * 128, :])
                    CW["w1t", h] = t
                t = cp.tile([128, 1], f32, tag="b1", name="b1c")
                nc.sync.dma_start(t[:], b1_col_d[:, :])
                CW["b1"] = t
                t = cp.tile([128, 128], bf16, tag="w2t", name="w2t")
                nc.sync.dma_start(t[:], W2T_d[:, :])
                CW["w2t"] = t
                t = cp.tile([128, 1], f32, tag="b2", name="b2c")
                nc.sync.dma_start(t[:], b2_col_d[:, :])
                CW["b2"] = t
                t = cp.tile([128, D_OUT], bf16, tag="w3t", name="w3t")
                nc.sync.dma_start(t[:], W3T_d[:, :])
                CW["w3t"] = t
                for h in range(2):
                    t = cp.tile([128, 1], f32, tag=f"b3{h}", name=f"b3c{h}")
                    nc.sync.dma_start(t[:], b3_col_d[h, :, :])
                    CW["b3", h] = t

            # ---- phase A with featT prefetch 2 loops ahead (cross-quarter) ----
            all_loops = [(q, r, ntl) for q in range(NQ) for r in range(NUM_REL)
                         for ntl in range(BUCKETS[q] // 512)]
            loop_of = {}
            for j, (q, r, ntl) in enumerate(all_loops):
                loop_of[q, r, ntl] = j
            ft_tiles = {}
            ft_next = [0]

            def ensure_ft(upto):
                while ft_next[0] <= min(upto, len(all_loops) - 1):
                    j = ft_next[0]
                    q, r, ntl = all_loops[j]
                    c0 = BUCKET_BASE[q] + ntl * 512
                    pair = []
                    for h in range(2):
                        t = wp.tile([128, 512], bf16, tag=f"ft{h}", bufs=3,
                                    name=f"ft{h}_{j}")
                        nc.sync.dma_start(
                            t[:], featT_d[h * 128:(h + 1) * 128, c0:c0 + 512])
                        pair.append(t)
                    ft_tiles[j] = pair
                    ft_next[0] += 1

            h_writes = [[] for _ in range(NQ)]

            def phase_a(q, rels=(0, 1)):
                """Build H table for bucket q, given relations, write to HBM."""
                for r in rels:
                    for ntl in range(BUCKETS[q] // 512):
                        j = loop_of[q, r, ntl]
                        ensure_ft(j + 2)
                        ft = ft_tiles.pop(j)
                        z1s = {}
                        for do_h in range(2):
                            z1p = psA.tile([128, 512], f32, tag="z1",
                                           space="PSUM", bufs=2)
                            for di_h in range(2):
                                nc.tensor.matmul(
                                    z1p[:],
                                    lhsT=W_rT_sb[r, di_h][:, do_h * 128:(do_h + 1) * 128],
                                    rhs=ft[di_h][:],
                                    start=(di_h == 0), stop=(di_h == 1))
                            # per-partition bias + relu + cast on the ACT unit
                            z = wp.tile([128, 512], bf16, tag=f"z1s{do_h}")
                            nc.scalar.activation(z[:], z1p[:], Relu,
                                                 bias=b_r_col_sb[r, do_h][:],
                                                 scale=1.0)
                            z1s[do_h] = z
                        hs = wp.tile([128, 4, D], bf16, tag="hs", bufs=2)
                        for cp2 in range(2):     # pairs of 128-node chunks
                            hp = psA.tile([128, 512], f32, tag="hp",
                                          space="PSUM", bufs=2)
                            for c4 in range(2):
                                sl = slice((cp2 * 2 + c4) * 128,
                                           (cp2 * 2 + c4 + 1) * 128)
                                dst_sl = slice(c4 * 256, (c4 + 1) * 256)
                                nc.tensor.matmul(hp[:, dst_sl],
                                                 lhsT=z1s[0][:, sl],
                                                 rhs=W_rT_sb[r, 0][:],
                                                 start=True, stop=False)
                                nc.tensor.matmul(hp[:, dst_sl],
                                                 lhsT=z1s[1][:, sl],
                                                 rhs=W_rT_sb[r, 1][:],
                                                 start=False, stop=True)
                            # bias lives on the free dim: DVE add (bf16 out),
                            # then ACT relu straight into the H staging tile
                            hpt = wp.tile([128, 512], bf16, tag="hpt", bufs=2)
                            nc.vector.tensor_tensor(out=hpt[:], in0=hp[:],
                                                    in1=b_r_rep2_sb[r][:],
                                                    op=Add)
                            nc.scalar.activation(hs[:, cp2 * 2:cp2 * 2 + 2, :],
                                                 hpt[:], Relu, bias=0.0,
                                                 scale=1.0)
                        w0 = (r * BUCKETS[q] + ntl * 512) // 128
                        w = nc.sync.dma_start(H_d[q][:, w0:w0 + 4, :], hs[:])
                        h_writes[q].append(w.ins)

            cnt_reg = nc.gpsimd.alloc_register("cnt_reg")

            def gather_sub(i, q, t0, tb):
                """One dma_gather sub-group; returns (msup, sel)."""
                sel_t = wp.tile([128, SUB, SELW], fp8, tag="sel", bufs=3,
                                name=f"sel{i}")
                nc.gpsimd.dma_start(sel_t[:, 0:tb, :], sel_d[:, t0:t0 + tb, :])
                msup = wp.tile([128, SUB, D], bf16, tag="msup", bufs=4)
                nc.gpsimd.reg_load(cnt_reg, cnts_sb[:, i:i + 1])
                g_inst = nc.gpsimd.dma_gather(
                    out_ap=msup[:, 0:tb, :],
                    in_ap=H_d[q][:, :, :].rearrange("p c d -> (p c) d"),
                    idxs_ap=idxs_sb[:, t0 * 8:(t0 + tb) * 8],
                    num_idxs=tb * 128, num_idxs_reg=cnt_reg,
                    elem_size=D, single_packet=False)
                for w in h_writes[q]:
                    add_dep_helper(g_inst.ins, w,
                                   reason="gather waits on Htab writes")
                return msup, sel_t

            def seg_sum(q, b):
                """Weighted segment-sum for group (q, b): aggT += msup^T @ sel."""
                first = (q == 0)
                pb = {}
                for fh in range(2):
                    pb[fh, 0] = psB.tile([128, 512], f32, tag="pb512",
                                         space="PSUM", bufs=2, name=f"pb512_{fh}")
                    pb[fh, 1] = psB.tile([128, 128], f32, tag="pb128",
                                         space="PSUM", bufs=2, name=f"pb128_{fh}")
                sub_list = subs_of[q, b]
                n_tiles = sum(tb for _, _, tb in sub_list)
                done = 0
                for i, t0, tb in sub_list:
                    msup, sel_t = gather_sub(i, q, t0, tb)
                    for t in range(tb):
                        st = (done + t == 0)
                        sp = (done + t == n_tiles - 1)
                        for fh in range(2):
                            lhsT = msup[:, t, fh * 128:(fh + 1) * 128]
                            nc.tensor.matmul(pb[fh, 0][:], lhsT=lhsT,
                                             rhs=sel_t[:, t, 0:512],
                                             start=st, stop=sp)
                            nc.tensor.matmul(pb[fh, 1][:], lhsT=lhsT,
                                             rhs=sel_t[:, t, 512:SELW],
                                             start=st, stop=sp)
                    done += tb
                for fh in range(2):
                    if first:
                        nc.vector.tensor_copy(aggT_sb[b, fh][:, 0:512],
                                              pb[fh, 0][:])
                        nc.vector.tensor_copy(aggT_sb[b, fh][:, 512:SELW],
                                              pb[fh, 1][:])
                    else:
                        nc.vector.tensor_add(aggT_sb[b, fh][:, 0:512],
                                             aggT_sb[b, fh][:, 0:512], pb[fh, 0][:])
                        nc.vector.tensor_add(aggT_sb[b, fh][:, 512:SELW],
                                             aggT_sb[b, fh][:, 512:SELW], pb[fh, 1][:])

            def phase_c(b):
                """LSTM step + MLP for block b, transposed layout, batched."""
                n0 = b * BLK
                nn = min(BLK, NPC - n0)
                aggT = {}
                for fh in range(2):
                    t = wp.tile([128, SELW], bf16, tag=f"aggb{fh}", bufs=2)
                    nc.vector.tensor_copy(t[:], aggT_sb[b, fh][:])
                    aggT[fh] = t
                for ck, (cs, cw) in enumerate(((0, 512), (512, 128))):
                    if cs >= nn:
                        continue
                    cw_v = min(cw, nn - cs)    # valid columns this chunk
                    gate_sb = {}
                    for gi, gname, fn in ((0, "i", Sig), (1, "g", Tanh),
                                          (2, "o", Sig)):
                        for gh in range(2):
                            gp = psA.tile([128, 512], f32, tag="z1",
                                          space="PSUM", bufs=2)
                            gc = gi * 2 + gh
                            for h in range(2):
                                nc.tensor.matmul(
                                    gp[:, 0:cw],
                                    lhsT=CW["wih", h][:, gc * 128:(gc + 1) * 128],
                                    rhs=aggT[h][:, cs:cs + cw],
                                    start=(h == 0), stop=(h == 1))
                            a = wp.tile([128, 512], f32, tag=f"gact{gh}", bufs=2)
                            nc.scalar.activation(a[:, 0:cw], gp[:, 0:cw], fn,
                                                 bias=CW["bg", gc][:], scale=1.0)
                            gate_sb[gname, gh] = a
                    hnT = {}
                    for gh in range(2):
                        cc = wp.tile([128, 512], f32, tag=f"cc{gh}", bufs=2)
                        nc.vector.tensor_mul(cc[:, 0:cw], gate_sb["i", gh][:, 0:cw],
                                             gate_sb["g", gh][:, 0:cw])
                        nc.scalar.activation(cc[:, 0:cw], cc[:, 0:cw], Tanh,
                                             bias=0.0, scale=1.0)
                        hn = wp.tile([128, 512], bf16, tag=f"hn{gh}", bufs=2)
                        nc.vector.tensor_mul(hn[:, 0:cw], gate_sb["o", gh][:, 0:cw],
                                             cc[:, 0:cw])
                        hnT[gh] = hn
                    # MLP: x1 = relu(W1 hn + b1), x2 = relu(W2 x1 + b2)
                    x1p = psA.tile([128, 512], f32, tag="z1",
                                   space="PSUM", bufs=2)
                    for h in range(2):
                        nc.tensor.matmul(x1p[:, 0:cw], lhsT=CW["w1t", h][:],
                                         rhs=hnT[h][:, 0:cw],
                                         start=(h == 0), stop=(h == 1))
                    x1s = wp.tile([128, 512], bf16, tag="x1s", bufs=2)
                    nc.scalar.activation(x1s[:, 0:cw], x1p[:, 0:cw], Relu,
                                         bias=CW["b1"][:], scale=1.0)
                    x2p = psA.tile([128, 512], f32, tag="z1",
                                   space="PSUM", bufs=2)
                    nc.tensor.matmul(x2p[:, 0:cw], lhsT=CW["w2t"][:],
                                     rhs=x1s[:, 0:cw], start=True, stop=True)
                    x2s = wp.tile([128, 512], bf16, tag="x2s", bufs=2)
                    nc.scalar.activation(x2s[:, 0:cw], x2p[:, 0:cw], Relu,
                                         bias=CW["b2"][:], scale=1.0)
                    for oh in range(2):
                        op = psA.tile([128, 512], f32, tag="z1",
                                      space="PSUM", bufs=2)
                        nc.tensor.matmul(op[:, 0:cw],
                                         lhsT=CW["w3t"][:, oh * 128:(oh + 1) * 128],
                                         rhs=x2s[:, 0:cw], start=True, stop=True)
                        osb = wp.tile([128, 512], f32, tag=f"osb{oh}", bufs=1)
                        nc.vector.tensor_scalar_add(
                            osb[:, 0:cw], op[:, 0:cw], CW["b3", oh][:])
                        nc.sync.dma_start(
                            outT_d[oh * 128:(oh + 1) * 128,
                                   n0 + cs:n0 + cs + cw_v],
                            osb[:, 0:cw_v])

            # ---- schedule: interleave phase A (producer) with phase B at
            # per-relation granularity so the in-order PE stream never traps
            # the H-table producer behind long consumer batches ----
            phase_a(0)
            phase_a(1)
            seg_sum(0, 0)
            phase_a(2, rels=(0,))
            seg_sum(0, 1)
            phase_a(2, rels=(1,))
            seg_sum(1, 0)
            phase_a(3, rels=(0,))
            seg_sum(1, 1)
            phase_a(3, rels=(1,))
            seg_sum(2, 0)
            phase_a(4, rels=(0,))
            seg_sum(2, 1)
            phase_a(4, rels=(1,))
            load_phase_c_weights()
            seg_sum(3, 0)
            seg_sum(3, 1)
            seg_sum(4, 0)
            phase_c(0)
            seg_sum(4, 1)
            phase_c(1)

    nc.compile()
    return nc


_CACHE = {}


def _get_nc(tiles_key):
    if tiles_key not in _CACHE:
        _CACHE[tiles_key] = _build([list(r) for r in tiles_key])
    return _CACHE[tiles_key]


# ----------------------------------------------------------------------------
# public entry
# ----------------------------------------------------------------------------

def kernel(**inputs) -> np.ndarray:
    src = np.asarray(inputs["src"], dtype=np.int32)
    dst = np.asarray(inputs["dst"], dtype=np.int32)
    rel = np.asarray(inputs["rel"], dtype=np.int32)

    com = _prep_weights(inputs)
    tiles_bq, idxs16_all, sel_all, cnts_all = _prep_edges(src, dst, rel)

    nc = _get_nc(tiles_bq)

    in_maps = []
    for c in range(NCORES):
        m = dict(com)
        m["idxs16"] = np.ascontiguousarray(idxs16_all[c])
        m["sel"] = np.ascontiguousarray(sel_all[c])
        m["cnts"] = np.ascontiguousarray(cnts_all[c])
        in_maps.append(m)

    res = bass_utils.run_bass_kernel_spmd(nc, in_maps, core_ids=list(range(NCORES)))

    out = np.empty((N_NODES, D_OUT), dtype=np.float32)
    for c in range(NCORES):
        out[c * NPC:(c + 1) * NPC, :] = res.results[c]["outT"].T
    return out
